# revision 1
# baseline (speedup 1.0000x reference)
"""Trainium2 Bass kernel for nn_BalancedLoss (composite segmentation loss).

Strategy: pure data-parallel over 8 NeuronCores (2 samples each). Each core
computes a small vector of partial reductions (sums / mins / maxes) over its
samples; the host combines them in float64, including the global min/max
normalization algebra for gradient_magnitude:

  mean((sa*a - oa - (sb*b - ob))^2) expands into moments
  E[a], E[a^2], E[b], E[b^2], E[ab]  (+ global min/max of a and b)

so a single device pass suffices despite the global min/max dependency.

On-device per 128-row window (9 overlapping windows per 1024-row sample):
  - All 3x3 convs are computed on the TensorEngine as 2-4 column-shifted
    accumulating matmuls with banded [128,128] matrices (vertical taps),
    so the full 3x3 conv result lands in PSUM with no vector-engine shifts.
  - Epilogues are fused DVE/ACT ops (tensor_scalar with abs_max/is_gt for
    thresholds, tensor_tensor_reduce with accumulate, activation with
    accum_out for softplus / sqrt-of-gradient-magnitude).
"""

import os
import numpy as np
from contextlib import ExitStack

DEBUG_NWIN = int(os.environ.get("KDBG_NWIN", "9"))

B, H, W = 16, 1024, 1024
NCORES = 8
SPC = B // NCORES  # samples per core
EPS = 1e-8
NPIX = H * W  # per-sample pixels
NTOT = B * NPIX

# window row-starts and valid partition bands [p0, p1)
WINDOWS = [(0, 0, 125)] + [(122 * w, 3, 125) for w in range(1, 8)] + [(896, 83, 128)]
NW_PER_SAMPLE = len(WINDOWS)
NWIN = SPC * NW_PER_SAMPLE  # accumulator columns

# accumulator quantity indices (each a [128, NWIN] block of the out tensor)
Q_SP, Q_SA, Q_SB = 0, 1, 2  # ACT-written accumulators (kept contiguous)
Q_TP, Q_EP = 3, 4
Q_GX2P, Q_GY2P, Q_MINP, Q_MAXP = 5, 6, 7, 8
Q_GX2D, Q_GY2D, Q_MIND, Q_MAXD = 9, 10, 11, 12
Q_AB, Q_CURV, Q_HGT = 13, 14, 15
Q_DSTAT = 16  # cols 0..3: sum(d) s0, sum(d^2) s0, sum(d) s1, sum(d^2) s1
NQ = 17
NQ_ACT = 3

FBIG = 3.0e38


def _tridiag(a, b, c, dtype):
    # out[p] = a*x[p-1] + b*x[p] + c*x[p+1] for matmul out = lhsT.T @ x
    M = np.zeros((128, 128), dtype=np.float64)
    idx = np.arange(128)
    M[idx, idx] = b
    M[idx[:-1], idx[1:]] = a  # row k=p-1, col p
    M[idx[1:], idx[:-1]] = c  # row k=p+1, col p
    return M.astype(dtype)


def _build_consts():
    import ml_dtypes
    bf16 = ml_dtypes.bfloat16
    mats = [
        _tridiag(1, 1, 1, bf16),                 # 0 M111
        _tridiag(1, 2, 1, bf16),                 # 1 M121
        _tridiag(-1, -2, -1, bf16),              # 2 -M121
        _tridiag(-1, 0, 1, bf16),                # 3 Mm101
        _tridiag(-2, 0, 2, bf16),                # 4 Mm202
        _tridiag(0, -9, 0, bf16),                # 5 -9I
        _tridiag(0, 1, 0, bf16),                 # 6 I
        _tridiag(1, -4, 1, bf16),                # 7 M1m41
        np.zeros((128, 2), bf16),                # zero pads source
    ]
    cbf16 = np.concatenate(mats, axis=1)  # [128, 8*128+2]
    return cbf16


_NC_CACHE = {}


def _build_nc():
    if "nc" in _NC_CACHE:
        return _NC_CACHE["nc"]
    import concourse.bass as bass
    import concourse.tile as tile
    import concourse.bass_isa as bass_isa
    from concourse import mybir

    fp32 = mybir.dt.float32
    bf16 = mybir.dt.bfloat16
    ALU = mybir.AluOpType
    ACTF = mybir.ActivationFunctionType

    nc = bass.Bass("TRN2", target_bir_lowering=False)
    pred_d = nc.declare_dram_parameter("pred", [SPC, H, W], fp32, isOutput=False)
    targ_d = nc.declare_dram_parameter("target", [SPC, H, W], fp32, isOutput=False)
    dem_d = nc.declare_dram_parameter("dem", [SPC, H, W], fp32, isOutput=False)
    cbf16_d = nc.declare_dram_parameter("cbf16", [128, 8 * 128 + 2], bf16,
                                        isOutput=False)
    ones_d = nc.declare_dram_parameter("onesf", [128, 128], fp32, isOutput=False)
    out_d = nc.declare_dram_parameter("out", [128, NQ * NWIN], fp32, isOutput=True)

    with tile.TileContext(nc) as tc:
        ctx = ExitStack()
        const = ctx.enter_context(tc.tile_pool(name="const", bufs=1))
        accp = ctx.enter_context(tc.tile_pool(name="accp", bufs=1))
        inp = ctx.enter_context(tc.tile_pool(name="inp", bufs=2))
        scr = ctx.enter_context(tc.tile_pool(name="scr", bufs=2))
        big = ctx.enter_context(tc.tile_pool(name="big", bufs=1))
        stp = ctx.enter_context(tc.tile_pool(name="stp", bufs=1))
        psum_v = ctx.enter_context(tc.tile_pool(name="psum_v", bufs=2, space="PSUM"))
        psum_a = ctx.enter_context(tc.tile_pool(name="psum_a", bufs=2, space="PSUM"))

        CB = const.tile([128, 8 * 128 + 2], bf16)
        def dma2(out_ap, in_ap, after=None):
            a = nc.sync.dma_start(out=out_ap[0:64], in_=in_ap[0:64])
            b = nc.sync.dma_start(out=out_ap[64:128], in_=in_ap[64:128])
            if after is not None:
                tile.add_dep_helper(a.ins, after.ins, sync=False,
                                    reason="order after absorber")
                tile.add_dep_helper(b.ins, after.ins, sync=False,
                                    reason="order after absorber")

        dma2(CB, cbf16_d[:, :])
        ONESF = const.tile([128, 128], fp32)
        dma2(ONESF, ones_d[:, :])
        EPSB = const.tile([128, 1], fp32)
        nc.gpsimd.memset(EPSB, EPS)
        NB10 = const.tile([128, 1], fp32)
        nc.gpsimd.memset(NB10, -10.0)
        Z2 = const.tile([128, 2], bf16)
        nc.gpsimd.memset(Z2, 0.0)
        ONE1 = const.tile([128, 1], fp32)
        nc.vector.memset(ONE1, 1.0)

        def mb(i):
            return CB[:, i * 128:(i + 1) * 128]

        M111B, M121B, M121NB, M101B, M202B, M9IB, IB, MLAPB = (
            mb(0), mb(1), mb(2), mb(3), mb(4), mb(5), mb(6), mb(7))
        ZPAD = cbf16_d[:, 8 * 128:8 * 128 + 2]

        # persistent double-buffered window tiles; pad columns zeroed once
        DBL = {}
        for par in (0, 1):
            DBL[("Tt", par)] = const.tile([128, 1024], fp32, name=f"Tt{par}")
            DBL[("Tp", par)] = const.tile([128, 1024], fp32, name=f"Tp{par}")
            DBL[("Td", par)] = const.tile([128, 1024], fp32, name=f"Td{par}")
            for nm in ("Ttb", "Tdb", "Tpp", "Te", "Tdl"):
                t = const.tile([128, 1026], bf16, name=f"{nm}{par}")
                DBL[(nm, par)] = t
                dma2(t[:, 0:1026:1025], ZPAD)

        # startup observers: each engine touches const/pad DMA queues once.
        # One PE accumulation group: only the first matmul carries waits beyond
        # its own rhs DMA queue, so each pad queue is observed with 1 wait.
        DOBS = psum_v.tile([128, 1024], fp32, tag="psv", name="DOBS")
        obs_rhs = [CB[:, 0:1]]
        for par in (0, 1):
            for nm in ("Ttb", "Tdb", "Tpp", "Te", "Tdl"):
                obs_rhs.append(DBL[(nm, par)][:, 0:1])
        n_obs = 2 * len(obs_rhs)
        i = 0
        for r in obs_rhs:
            for h in (slice(0, 64), slice(64, 128)):
                nc.tensor.matmul(DOBS[0:64, 0:1], CB[h, 0:64], r[h],
                                 start=(i == 0), stop=(i == n_obs - 1))
                i += 1
        DUMS = const.tile([1, 1], fp32, name="dums")
        DUMD = const.tile([1, 24], fp32, name="dumd")
        nc.vector.memset(DUMS, 0.0)
        OBSA = [const.tile([128, 2], bf16, name=f"obsa{p}") for p in (0, 1, 2)]
        OBSV = [const.tile([128, 2], fp32, name=f"obsv{p}{n}")
                for p in (0, 1) for n in range(5)]
        nc.scalar.activation(out=OBSA[2], in_=Z2, func=ACTF.Identity, bias=0.0)
        k = 0
        for par in (0, 1):
            for nm in ("Ttb", "Tdb", "Tpp", "Te", "Tdl"):
                t = DBL[(nm, par)]
                for h in (slice(0, 64), slice(64, 128)):
                    if nm == "Tpp":
                        nc.scalar.activation(out=OBSA[par][h, 0:1],
                                             in_=t[h, 0:1], func=ACTF.Copy)
                    else:
                        nc.vector.tensor_scalar(out=OBSV[k][h, 0:1],
                                                in0=t[h, 0:1], scalar1=1.0,
                                                scalar2=None, op0=ALU.mult)
                k += 1

        # one accumulator tile; per-quantity views. ACT-written quantities
        # accumulate into a side tile that DVE mirrors in, so the single
        # out-store depends on DVE alone (1-wait HWDGE slot).
        ACCBIG = accp.tile([128, NQ * NWIN], fp32, name="accbig")
        ACTACC = accp.tile([128, NQ_ACT * NWIN], fp32, name="actacc")
        ACC = [ACTACC[:, q * NWIN:(q + 1) * NWIN] if q < NQ_ACT
               else ACCBIG[:, q * NWIN:(q + 1) * NWIN] for q in range(NQ)]

        # ---------------- dem stats prepass (per sample) ----------------
        DS = stp.tile([128, 4], fp32)
        for s in range(SPC):
            sdmy = None
            if s > 0:
                sdmy = nc.sync.nop()
                tile.add_dep_helper(sdmy.ins, stats_last.ins, sync=True,
                                    reason="absorb stats reader WAR")
            demfull = big.tile([128, 8, 1024], fp32, tag="demfull")
            src = dem_d[s, :, :].rearrange("(b p) w -> p b w", p=128)
            for q4 in range(4):
                dma2(demfull[:, 2 * q4:2 * q4 + 2, :],
                     src[:, 2 * q4:2 * q4 + 2, :], after=sdmy)
            DS2 = stp.tile([128, 8], fp32, tag=f"ds2_{s}", name=f"ds2_{s}")
            sq = big.tile([128, 8 * 1024], bf16, tag="sqscr")
            for q4 in range(4):
                for h in (slice(0, 64), slice(64, 128)):
                    nc.vector.tensor_reduce(
                        out=DS2[h, q4:q4 + 1],
                        in_=demfull[h, 2 * q4:2 * q4 + 2, :],
                        axis=mybir.AxisListType.XY, op=ALU.add)
                    dh = demfull[h, 2 * q4:2 * q4 + 2, :].rearrange(
                        "p b w -> p (b w)")
                    nc.vector.scalar_tensor_tensor(
                        out=sq[h, 2048 * q4:2048 * (q4 + 1)], in0=dh, scalar=1.0,
                        in1=dh, op0=ALU.mult, op1=ALU.mult,
                        accum_out=DS2[h, 4 + q4:5 + q4])
            nc.vector.tensor_tensor(out=DS2[:, 0:1], in0=DS2[:, 0:1],
                                    in1=DS2[:, 1:2], op=ALU.add)
            nc.vector.tensor_tensor(out=DS2[:, 2:3], in0=DS2[:, 2:3],
                                    in1=DS2[:, 3:4], op=ALU.add)
            nc.vector.tensor_tensor(out=DS[:, 2 * s:2 * s + 1], in0=DS2[:, 0:1],
                                    in1=DS2[:, 2:3], op=ALU.add)
            nc.vector.tensor_tensor(out=DS2[:, 4:5], in0=DS2[:, 4:5],
                                    in1=DS2[:, 5:6], op=ALU.add)
            nc.vector.tensor_tensor(out=DS2[:, 6:7], in0=DS2[:, 6:7],
                                    in1=DS2[:, 7:8], op=ALU.add)
            stats_last = nc.vector.tensor_tensor(
                out=DS[:, 2 * s + 1:2 * s + 2],
                in0=DS2[:, 4:5], in1=DS2[:, 6:7], op=ALU.add)
        PSW = psum_v.tile([128, 1024], fp32, tag="psv", name="PSstats")
        nc.tensor.matmul(PSW[:, 0:4], ONESF, DS, start=True, stop=True)
        PS = stp.tile([128, 4], fp32)
        nc.vector.tensor_scalar(out=PS, in0=PSW[:, 0:4], scalar1=1.0,
                                scalar2=None, op0=ALU.mult)
        # per-sample scalar chain -> inv = 1/(std+EPS), nb = -mean*inv
        ST = stp.tile([128, 16], fp32)
        for s in range(SPC):
            c = 8 * s
            mu = ST[:, c:c + 1]
            ex2 = ST[:, c + 1:c + 2]
            m2 = ST[:, c + 2:c + 3]
            vr = ST[:, c + 3:c + 4]
            sd = ST[:, c + 4:c + 5]
            sde = ST[:, c + 5:c + 6]
            inv = ST[:, c + 6:c + 7]
            nb = ST[:, c + 7:c + 8]
            nc.vector.tensor_scalar(out=mu, in0=PS[:, 2 * s:2 * s + 1],
                                    scalar1=1.0 / NPIX, scalar2=None, op0=ALU.mult)
            nc.vector.tensor_scalar(out=ex2, in0=PS[:, 2 * s + 1:2 * s + 2],
                                    scalar1=1.0 / NPIX, scalar2=None, op0=ALU.mult)
            nc.vector.tensor_tensor(out=m2, in0=mu, in1=mu, op=ALU.mult)
            nc.vector.tensor_tensor(out=vr, in0=ex2, in1=m2, op=ALU.subtract)
            nc.scalar.activation(out=sd, in_=vr, func=ACTF.Sqrt,
                                 scale=float(NPIX) / (NPIX - 1))
            nc.vector.tensor_scalar(out=sde, in0=sd, scalar1=EPS, scalar2=None,
                                    op0=ALU.add)
            nc.vector.reciprocal(out=inv, in_=sde)
            nc.vector.scalar_tensor_tensor(out=nb, in0=mu, scalar=-1.0, in1=inv,
                                           op0=ALU.mult, op1=ALU.mult)

        # ---------------- main windows ----------------
        def conv(ps, groups, srctile):
            for c0 in (0, 512):
                for i, (mat, dx) in enumerate(groups):
                    nc.tensor.matmul(
                        ps[:, c0:c0 + 512], mat,
                        srctile[:, c0 + dx + 1:c0 + dx + 1 + 512],
                        start=(i == 0), stop=(i == len(groups) - 1))

        last_dve = {}
        for s in range(SPC):
            c8 = 8 * s
            inv_ap = ST[:, c8 + 6:c8 + 7]
            nb_ap = ST[:, c8 + 7:c8 + 8]
            for wi, (r0, p0, p1) in enumerate(WINDOWS[:DEBUG_NWIN]):
                wcol = s * NW_PER_SAMPLE + wi
                par = (s * NW_PER_SAMPLE + wi) % 2
                Tt, Tp, Td = DBL[("Tt", par)], DBL[("Tp", par)], DBL[("Td", par)]
                Ttb, Tdb = DBL[("Ttb", par)], DBL[("Tdb", par)]
                Tpp, Te, Tdl = DBL[("Tpp", par)], DBL[("Te", par)], DBL[("Tdl", par)]
                gw = s * NW_PER_SAMPLE + wi
                dmy = None
                if gw >= 2:
                    dmy = nc.sync.nop()
                    for gwp in (gw - 2, gw - 1):
                        for rd in last_dve.get(gwp, []):
                            tile.add_dep_helper(dmy.ins, rd.ins, sync=True,
                                                reason="absorb reader WAR")
                dma2(Tt, targ_d[s, r0:r0 + 128, :], after=dmy)
                dma2(Tp, pred_d[s, r0:r0 + 128, :], after=dmy)
                dma2(Td, dem_d[s, r0:r0 + 128, :], after=dmy)

                # first-touch converts (keep matmuls off DMA-queue sems);
                # per partition-half so each op sees one DMA queue
                rdrs = []
                Tpf = scr.tile([128, 1024], fp32, tag="Tpf")
                for h in (slice(0, 64), slice(64, 128)):
                    rdrs.append(nc.vector.tensor_scalar(
                        out=Ttb[h, 1:1025], in0=Tt[h], scalar1=1.0,
                        scalar2=None, op0=ALU.mult))
                    rdrs.append(nc.vector.tensor_scalar(
                        out=Tdb[h, 1:1025], in0=Td[h], scalar1=1.0,
                        scalar2=None, op0=ALU.mult))
                    rdrs.append(nc.vector.tensor_scalar(
                        out=Tpf[h], in0=Tp[h], scalar1=1.0,
                        scalar2=None, op0=ALU.mult))

                # ---- target edge chain ----
                bx = psum_v.tile([128, 1024], fp32, tag="psv")
                conv(bx, [(M111B, -1), (M111B, 0), (M111B, 1), (M9IB, 0)], Ttb)
                e1 = scr.tile([128, 1024], bf16, tag="e1")
                nc.vector.tensor_scalar(out=e1, in0=bx, scalar1=1.35,
                                        scalar2=None, op0=ALU.is_gt)
                e2 = scr.tile([128, 1024], bf16, tag="e2")
                nc.vector.tensor_scalar(out=e2, in0=bx, scalar1=-1.35,
                                        scalar2=None, op0=ALU.is_lt)
                nc.vector.tensor_tensor(out=Te[:, 1:1025], in0=e1, in1=e2,
                                        op=ALU.add)
                dl = psum_v.tile([128, 1024], fp32, tag="psv")
                conv(dl, [(M111B, -1), (M111B, 0), (M111B, 1)], Te)
                nc.vector.tensor_scalar(out=Tdl[:, 1:1025], in0=dl, scalar1=0.5,
                                        scalar2=None, op0=ALU.is_gt)
                er = psum_v.tile([128, 1024], fp32, tag="psv")
                conv(er, [(M111B, -1), (M111B, 0), (M111B, 1)], Tdl)
                Et = scr.tile([128, 1024], bf16, tag="Et")
                nc.vector.tensor_scalar(out=Et, in0=er, scalar1=8.5,
                                        scalar2=None, op0=ALU.is_gt)

                # ---- bce partials ----
                ex1 = scr.tile([128, 1024], fp32, tag="ex1")
                nc.scalar.activation(out=ex1, in_=Tpf, func=ACTF.Exp)
                sps = scr.tile([128, 1024], bf16, tag="sps")
                nc.scalar.activation(out=sps, in_=ex1, func=ACTF.Ln, bias=1.0,
                                     accum_out=ACC[Q_SP][:, wcol:wcol + 1])
                s1 = scr.tile([128, 1024], fp32, tag="ttrscr")
                rdrs.append(nc.vector.scalar_tensor_tensor(out=s1, in0=Tt, scalar=1.0, in1=Tp, op0=ALU.mult, op1=ALU.mult, accum_out=ACC[Q_TP][:, wcol:wcol + 1]))
                s2 = scr.tile([128, 1024], fp32, tag="ttrscr")
                rdrs.append(nc.vector.scalar_tensor_tensor(out=s2, in0=Et, scalar=1.0, in1=Tp, op0=ALU.mult, op1=ALU.mult, accum_out=ACC[Q_EP][:, wcol:wcol + 1]))

                # ---- pred prob + sobel ----
                ppt = scr.tile([128, 1024], bf16, tag="ppt")
                nc.scalar.activation(out=ppt, in_=Tpf, func=ACTF.Sigmoid)
                nc.vector.tensor_scalar(out=Tpp[:, 1:1025], in0=ppt, scalar1=1.0,
                                        scalar2=None, op0=ALU.mult)

                def grad_mag(srctile, act_evac, qgx2, qgy2, qsa, qmin, qmax):
                    pool = psum_a if act_evac else psum_v
                    tg = "psa" if act_evac else "psv"
                    gx = pool.tile([128, 1024], fp32, tag=tg)
                    conv(gx, [(M121NB, -1), (M121B, 1)], srctile)
                    gy = pool.tile([128, 1024], fp32, tag=tg)
                    conv(gy, [(M101B, -1), (M101B, 1), (M202B, 0)], srctile)
                    gx2 = scr.tile([128, 1024], bf16, tag="g2a")
                    gy2 = scr.tile([128, 1024], bf16, tag="g2b")
                    if act_evac:
                        nc.scalar.activation(out=gx2, in_=gx, func=ACTF.Square,
                                             accum_out=ACC[qgx2][:, wcol:wcol + 1])
                        nc.scalar.activation(out=gy2, in_=gy, func=ACTF.Square,
                                             accum_out=ACC[qgy2][:, wcol:wcol + 1])
                    else:
                        gxe = scr.tile([128, 1024], bf16, tag="gxe")
                        nc.vector.tensor_scalar(out=gxe, in0=gx, scalar1=1.0,
                                                scalar2=None, op0=ALU.mult)
                        gye = scr.tile([128, 1024], bf16, tag="gye")
                        nc.vector.tensor_scalar(out=gye, in0=gy, scalar1=1.0,
                                                scalar2=None, op0=ALU.mult)
                        nc.vector.scalar_tensor_tensor(
                            out=gx2, in0=gxe, scalar=1.0, in1=gxe,
                            op0=ALU.mult, op1=ALU.mult,
                            accum_out=ACC[qgx2][:, wcol:wcol + 1])
                        nc.vector.scalar_tensor_tensor(
                            out=gy2, in0=gye, scalar=1.0, in1=gye,
                            op0=ALU.mult, op1=ALU.mult,
                            accum_out=ACC[qgy2][:, wcol:wcol + 1])
                    g2 = scr.tile([128, 1024], bf16, tag="g2c")
                    nc.vector.tensor_tensor(out=g2, in0=gx2, in1=gy2, op=ALU.add)
                    nc.vector.tensor_reduce(out=ACC[qmin][:, wcol:wcol + 1], in_=g2, axis=mybir.AxisListType.X, op=ALU.min)
                    nc.vector.tensor_reduce(
                        out=ACC[qmax][:, wcol:wcol + 1], in_=g2,
                        axis=mybir.AxisListType.X, op=ALU.max)
                    av = scr.tile([128, 1024], bf16, tag=f"av{qsa}")
                    nc.scalar.activation(out=av, in_=g2, func=ACTF.Sqrt,
                                         bias=EPSB,
                                         accum_out=ACC[qsa][:, wcol:wcol + 1])
                    return av

                av_p = grad_mag(Tpp, False, Q_GX2P, Q_GY2P, Q_SA, Q_MINP, Q_MAXP)
                av_d = grad_mag(Tdb, False, Q_GX2D, Q_GY2D, Q_SB, Q_MIND, Q_MAXD)
                s3 = scr.tile([128, 1024], fp32, tag="ttrscr")
                nc.vector.scalar_tensor_tensor(out=s3, in0=av_p, scalar=1.0, in1=av_d, op0=ALU.mult, op1=ALU.mult, accum_out=ACC[Q_AB][:, wcol:wcol + 1])

                # ---- curvature ----
                lp = psum_v.tile([128, 1024], fp32, tag="psv")
                conv(lp, [(IB, -1), (IB, 1), (MLAPB, 0)], Tdb)
                lpe = scr.tile([128, 1024], bf16, tag="lpe")
                nc.vector.tensor_scalar(out=lpe, in0=lp, scalar1=1.0,
                                        scalar2=None, op0=ALU.mult)
                c1 = scr.tile([128, 1024], bf16, tag="c1")
                nc.scalar.activation(out=c1, in_=lpe, func=ACTF.Sigmoid,
                                     scale=0.2)
                cs = scr.tile([128, 1024], bf16, tag="cs")
                nc.scalar.activation(out=cs, in_=c1, func=ACTF.Sigmoid,
                                     scale=20.0, bias=NB10)
                s4 = scr.tile([128, 1024], fp32, tag="ttrscr")
                nc.vector.scalar_tensor_tensor(
                    out=s4, in0=Tpp[:, 1:1025], scalar=1.0, in1=cs,
                    op0=ALU.mult, op1=ALU.mult,
                    accum_out=ACC[Q_CURV][:, wcol:wcol + 1])

                # ---- height norm ----
                zt = scr.tile([128, 1024], bf16, tag="zt")
                for h in (slice(0, 64), slice(64, 128)):
                    rdrs.append(nc.vector.tensor_scalar(
                        out=zt[h], in0=Td[h], scalar1=inv_ap[h],
                        scalar2=nb_ap[h], op0=ALU.mult, op1=ALU.add))
                z2 = scr.tile([128, 1024], bf16, tag="z2")
                nc.vector.tensor_tensor(out=z2, in0=zt, in1=zt,
                                        op=ALU.mult)
                hn = scr.tile([128, 1024], bf16, tag="hn")
                nc.scalar.activation(out=hn, in_=z2, func=ACTF.Exp,
                                     scale=-0.5)
                s5 = scr.tile([128, 1024], fp32, tag="ttrscr")
                rdrs.append(nc.vector.scalar_tensor_tensor(
                    out=s5, in0=Tpp[:, 1:1025], scalar=1.0, in1=hn,
                    op0=ALU.mult, op1=ALU.mult,
                    accum_out=ACC[Q_HGT][:, wcol:wcol + 1]))
                last_dve[gw] = rdrs

        nc.vector.tensor_copy(out=ACC[Q_DSTAT][:, 0:4], in_=DS)

        nc.vector.tensor_scalar(out=ACCBIG[:, 0:NQ_ACT * NWIN], in0=ACTACC,
                                scalar1=1.0, scalar2=None, op0=ALU.mult)
        nc.sync.dma_start(out=out_d[:, :], in_=ACCBIG[:, :])
        ctx.close()
    # Walrus's codegen allows only ONE sync-wait per instruction. Tile's
    # emission is per-instruction structural and does not exploit same-engine
    # transitivity, so we minimize here:
    #  1. replay each engine's stream tracking observed (sem -> max threshold);
    #     waits at or below an already-observed threshold are implied.
    #  2. same-engine self-sem waits are implied by program order (per-op
    #     drain on DVE/ACT; pc-monotone completion on PE).
    #  3. a DMA's lane-WAW wait on a prior same-tile load is implied by the
    #     reader-WAR wait that remains (readers gate the overwrite).
    ENG_SEM = {"PE": "PE", "DVE": "DVE", "Activation": "Activation",
               "Pool": "Pool", "SP": "SP_sequencer"}
    observed = {}  # (engine, sem_name) -> threshold
    # sems that are ever decremented (barrier gathers) are not monotone;
    # never treat their waits as implied.
    nonmono = set()
    for blk in nc.m.functions[0].blocks:
        for ins in blk.instructions:
            if ins.sync_info is None:
                continue
            for u in ins.sync_info.on_update:
                um = str(u.update_mode)
                if "sub" in um or "dec" in um:
                    nonmono.add(u.ant_name)

    def eng_of(ins):
        e = getattr(ins, "engine", None)
        return str(e).split(".")[-1] if e is not None else "SP"

    for blk in nc.m.functions[0].blocks:
        for ins in blk.instructions:
            si = ins.sync_info
            if si is None:
                continue
            eng = eng_of(ins)
            ws = list(si.on_wait)
            if not ws:
                continue
            # 1. drop implied-by-observation waits
            kept = []
            for w in ws:
                if w.ant_name not in nonmono and \
                        str(w.wait_mode) == "sem-ge-imm" and \
                        observed.get((eng, w.ant_name), -1) >= w.wait_value:
                    continue
                kept.append(w)
            # 2. drop self-engine waits if still over budget
            if len(kept) > 1:
                self_sem = ENG_SEM.get(eng, "zz")
                non_self = [w for w in kept
                            if not w.ant_name.startswith(self_sem)]
                if non_self:
                    kept = non_self
            # 3b. matmul: PE waits implied by pc-monotone completion
            if len(kept) > 1 and type(ins).__name__ == "InstMatmult":
                nonpe = [w for w in kept if not w.ant_name.startswith("PE")]
                kept = nonpe if nonpe else kept[:1]
            si.on_wait = kept
            for w in kept:
                if w.ant_name in nonmono:
                    continue
                k = (eng, w.ant_name)
                observed[k] = max(observed.get(k, -1), w.wait_value)

    # stragglers: drop PE WAR waits (covered by intervening evac chains)
    for blk in nc.m.functions[0].blocks:
        for ins in blk.instructions:
            si = ins.sync_info
            if si is None or len(si.on_wait) <= 1:
                continue
            if type(ins).__name__ != "InstDrain":
                nonpe = [w for w in si.on_wait
                         if not w.ant_name.startswith("PE")]
                si.on_wait = nonpe if nonpe else si.on_wait[:1]

    # final out-store: its DVE wait places it after all compute (and hence
    # after every lane peer), so the lane-order wait is redundant.
    all_dmas = [ins for blk in nc.m.functions[0].blocks
                for ins in blk.instructions
                if type(ins).__name__ == "InstDMACopy"]
    if all_dmas:
        fin = all_dmas[-1]
        if fin.sync_info and len(fin.sync_info.on_wait) > 1:
            eng = [w for w in fin.sync_info.on_wait
                   if not w.ant_name.startswith(("DMAHW", "DMASW"))]
            if eng:
                fin.sync_info.on_wait = eng

    # tail drain: engine completion is implied by the final store's waits;
    # wait only on the store's DMA lane.
    out_dmas = [ins for blk in nc.m.functions[0].blocks
                for ins in blk.instructions
                if type(ins).__name__ == "InstDMACopy"][-1:]
    keep_lanes = set()
    for ins in out_dmas:
        for u in (ins.sync_info.on_update if ins.sync_info else []):
            if u.ant_name.startswith(("DMAHW", "DMASW")):
                keep_lanes.add(u.ant_name)
    for blk in nc.m.functions[0].blocks:
        for ins in blk.instructions:
            if type(ins).__name__ == "InstDrain" and ins.sync_info and \
                    len(ins.sync_info.on_wait) > 1:
                lane_ws = [w for w in ins.sync_info.on_wait
                           if w.ant_name in keep_lanes]
                if lane_ws:
                    ins.sync_info.on_wait = lane_ws
    # this walrus build can't encode EVENT_SEMAPHORE_RANGE_CLEAR (InstISA 176)
    # emitted by the sem-pool free; drop it (single-shot NEFF execution).
    for blk in nc.m.functions[0].blocks:
        bad = [i for i, ins in enumerate(blk.instructions)
               if type(ins).__name__ == "InstISA"]
        if bad:
            keep = [ins for ins in blk.instructions
                    if type(ins).__name__ != "InstISA"]
            try:
                blk.instructions = keep
            except Exception:
                for i in reversed(bad):
                    del blk.instructions[i]
    _NC_CACHE["nc"] = nc
    return nc


def _band_mask():
    """[128, NWIN] mask selecting each window's valid (non-overlap) rows."""
    m = np.zeros((128, NWIN))
    for s in range(SPC):
        for wi, (r0, p0, p1) in enumerate(WINDOWS):
            m[p0:p1, s * NW_PER_SAMPLE + wi] = 1.0
    return m


def _combine(outs):
    """outs: list of [128, NQ*NWIN] f32 arrays (one per core). float64 combine."""
    A = np.stack([o.reshape(128, NQ, NWIN).astype(np.float64) for o in outs])
    m = _band_mask()[None, :, None, :]  # [1, 128, 1, NWIN]
    sums = (A * m).sum(axis=(0, 1, 3))  # per quantity (valid for sum quantities)
    s_sp, s_tp, s_ep = sums[Q_SP], sums[Q_TP], sums[Q_EP]
    s_gx2p, s_gy2p, s_a = sums[Q_GX2P], sums[Q_GY2P], sums[Q_SA]
    s_gx2d, s_gy2d, s_b = sums[Q_GX2D], sums[Q_GY2D], sums[Q_SB]
    s_ab, s_curv, s_hgt = sums[Q_AB], sums[Q_CURV], sums[Q_HGT]
    mm = m[:, :, 0, :]
    min_g2p = np.where(mm > 0, A[:, :, Q_MINP, :], FBIG).min()
    max_g2p = np.where(mm > 0, A[:, :, Q_MAXP, :], -FBIG).max()
    min_g2d = np.where(mm > 0, A[:, :, Q_MIND, :], FBIG).min()
    max_g2d = np.where(mm > 0, A[:, :, Q_MAXD, :], -FBIG).max()

    bce1 = (s_sp - s_tp) / NTOT
    bce2 = (s_sp - s_ep) / NTOT

    e_a2 = (s_gx2p + s_gy2p) / NTOT + EPS
    e_b2 = (s_gx2d + s_gy2d) / NTOT + EPS
    amin, amax = np.sqrt(min_g2p + EPS), np.sqrt(max_g2p + EPS)
    bmin, bmax = np.sqrt(min_g2d + EPS), np.sqrt(max_g2d + EPS)

    def scale_off(lo, hi):
        if hi > lo:
            s = 1.0 / (hi - lo + EPS)
            return s, lo * s
        return 1.0, 0.0

    sa, oa = scale_off(amin, amax)
    sb, ob = scale_off(bmin, bmax)
    cc = oa - ob
    e_a, e_b, e_ab = s_a / NTOT, s_b / NTOT, s_ab / NTOT
    grad_cons = (sa * sa * e_a2 + sb * sb * e_b2 + cc * cc
                 - 2.0 * sa * sb * e_ab - 2.0 * cc * sa * e_a + 2.0 * cc * sb * e_b)

    height_cons = -s_hgt / NTOT
    curv_cons = -s_curv / NTOT
    geo = grad_cons + 0.5 * height_cons + 0.3 * curv_cons
    total = 0.8 * bce1 + 0.1 * bce2 + 0.1 * geo
    return np.float32(total)


_CONSTS = {}
_ONES128 = np.ones((128, 128), np.float32)


def kernel(pred, target, dem, _profile=False):
    from concourse.bass_utils import run_bass_kernel_spmd

    if "c" not in _CONSTS:
        _CONSTS["c"] = _build_consts()
    cbf16 = _CONSTS["c"]
    nc = _build_nc()

    p = np.ascontiguousarray(pred.reshape(B, H, W), dtype=np.float32)
    t = np.ascontiguousarray(target.reshape(B, H, W), dtype=np.float32)
    d = np.ascontiguousarray(dem.reshape(B, H, W), dtype=np.float32)
    in_maps = []
    for c in range(NCORES):
        sl = slice(c * SPC, (c + 1) * SPC)
        in_maps.append({
            "pred": p[sl], "target": t[sl], "dem": d[sl],
            "cbf16": cbf16, "onesf": _ONES128,
        })
    res = run_bass_kernel_spmd(nc, in_maps, core_ids=list(range(NCORES)),
                               trace=_profile)
    outs = [m["out"] for m in res.results]
    total = _combine(outs)
    if _profile:
        return total, res
    return total



# revision 54
# speedup vs baseline: 1.1231x; 1.1231x over previous
"""Trainium2 Bass kernel for nn_BalancedLoss (composite segmentation loss).

Data-parallel over 8 NeuronCores (2 samples each). Each core emits a
[128, NQ*NWIN] tensor of per-window partial reductions; the host combines
them in float64 (global min/max normalization handled via moment algebra).

v3 restructure vs baseline (915us):
  - No dem-stats prepass: dem sum/sumsq accumulate during the main windows,
    per-sample mean/std finalized on device, and the height-norm term runs
    as a second pass over SBUF-resident bf16 sigmoid(pred)/dem (no extra HBM
    traffic).
  - Engine rebalance within ISA limits: Pool (GPSIMD) takes product tiles /
    g2 adds (tensor_tensor, SBUF-only); DVE does thresholds, reductions and
    cheap 4x-mode accumulate-sums of the Pool product tiles; ACT does
    sigmoid/ln/sqrt/square ordered to minimize ACT_TABLE_LOADs, with
    softplus folded into -ln(1-sigmoid(p)) to reuse the sigmoid tile.
  - Whole-tile DMAs (one HWDGE lane per tile) so full-width consumers carry
    a single wait; the graph is engineered so every instruction needs at
    most ONE hardware sync-wait (walrus limit).
"""

import os
import numpy as np
from contextlib import ExitStack

B, H, W = 16, 1024, 1024
NCORES = 8
SPC = B // NCORES  # samples per core
EPS = 1e-8
NPIX = H * W
NTOT = B * NPIX

# window row-starts and valid partition bands [p0, p1)
WINDOWS = [(0, 0, 125)] + [(122 * w, 3, 125) for w in range(1, 8)] + [(896, 83, 128)]
NW = len(WINDOWS)
NWIN = SPC * NW

# quantity indices: [0..3) ACT-written, [3..16) DVE-written
Q_SP, Q_SA, Q_SB = 0, 1, 2  # Q_SP holds sum(ln(1-pp)) = -sum(softplus(p))
Q_G2P, Q_G2D, Q_MAXP, Q_MINP, Q_MAXD, Q_MIND = 3, 4, 5, 6, 7, 8
Q_TP, Q_EP, Q_DSQ, Q_AB, Q_CURV, Q_HGT, Q_DSUM = 9, 10, 11, 12, 13, 14, 15
NQ_ACT = 3
NQ = 16

FBIG = 3.0e38


def _tridiag(a, b, c, dtype):
    # out[p] = a*x[p-1] + b*x[p] + c*x[p+1] for matmul out = lhsT.T @ x
    M = np.zeros((128, 128), dtype=np.float64)
    idx = np.arange(128)
    M[idx, idx] = b
    M[idx[:-1], idx[1:]] = a  # row k=p-1, col p
    M[idx[1:], idx[:-1]] = c  # row k=p+1, col p
    return M.astype(dtype)


def _build_consts():
    import ml_dtypes
    bf16 = ml_dtypes.bfloat16
    mats = [
        _tridiag(1, 1, 1, bf16),                 # 0 M111
        _tridiag(1, 2, 1, bf16),                 # 1 M121
        _tridiag(-1, -2, -1, bf16),              # 2 -M121
        _tridiag(-1, 0, 1, bf16),                # 3 Mm101
        _tridiag(-2, 0, 2, bf16),                # 4 Mm202
        _tridiag(0, -9, 0, bf16),                # 5 -9I
        _tridiag(0, 1, 0, bf16),                 # 6 I
        _tridiag(1, -4, 1, bf16),                # 7 M1m41
    ]
    return np.concatenate(mats, axis=1)  # [128, 8*128]


def _band_mask9():
    m = np.zeros((128, NW), np.float32)
    for wi, (r0, p0, p1) in enumerate(WINDOWS):
        m[p0:p1, wi] = 1.0
    return m


_NC_CACHE = {}


def _build_nc():
    if "nc" in _NC_CACHE:
        return _NC_CACHE["nc"]
    import concourse.bass as bass
    import concourse.tile as tile
    from concourse import mybir

    fp32 = mybir.dt.float32
    bf16 = mybir.dt.bfloat16
    ALU = mybir.AluOpType
    ACTF = mybir.ActivationFunctionType
    AXL = mybir.AxisListType

    nc = bass.Bass("TRN2", target_bir_lowering=False)
    pred_d = nc.declare_dram_parameter("pred", [SPC, H, W], fp32, isOutput=False)
    targ_d = nc.declare_dram_parameter("target", [SPC, H, W], fp32, isOutput=False)
    dem_d = nc.declare_dram_parameter("dem", [SPC, H, W], fp32, isOutput=False)
    cbf16_d = nc.declare_dram_parameter("cbf16", [128, 8 * 128], bf16,
                                        isOutput=False)
    ones_d = nc.declare_dram_parameter("onesf", [128, 128], fp32, isOutput=False)
    bmask_d = nc.declare_dram_parameter("bmask", [128, NW], fp32, isOutput=False)
    out_d = nc.declare_dram_parameter("out", [128, NQ * NWIN], fp32, isOutput=True)

    with tile.TileContext(nc) as tc:
        ctx = ExitStack()
        const = ctx.enter_context(tc.tile_pool(name="const", bufs=1))
        accp = ctx.enter_context(tc.tile_pool(name="accp", bufs=1))
        scr = ctx.enter_context(tc.tile_pool(name="scr", bufs=2))
        pse = ctx.enter_context(tc.tile_pool(name="pse", bufs=1, space="PSUM"))
        psl = ctx.enter_context(tc.tile_pool(name="psl", bufs=1, space="PSUM"))
        pss = ctx.enter_context(tc.tile_pool(name="pss", bufs=2, space="PSUM"))

        # ---- consts (3 DMAs -> 3 HWDGE lanes) ----
        CB = const.tile([128, 8 * 128], bf16)
        nc.sync.dma_start(out=CB, in_=cbf16_d[:, :])
        ONESF = const.tile([128, 128], fp32)
        nc.sync.dma_start(out=ONESF, in_=ones_d[:, :])
        BMASK = const.tile([128, NW], fp32)
        nc.sync.dma_start(out=BMASK, in_=bmask_d[:, :])

        EPSB = const.tile([128, 1], fp32)
        msets = [nc.gpsimd.memset(EPSB, EPS)]

        def mb(i):
            return CB[:, i * 128:(i + 1) * 128]

        M111B, M121B, M121NB, M101B, M202B, M9IB, IB, MLAPB = (
            mb(0), mb(1), mb(2), mb(3), mb(4), mb(5), mb(6), mb(7))

        # ---- persistent tiles ----
        TT = [const.tile([128, 1024], fp32, name=f"TT{p}") for p in (0, 1)]
        TP = [const.tile([128, 1024], fp32, name=f"TP{p}") for p in (0, 1)]
        TD = [const.tile([128, 1024], fp32, name=f"TD{p}") for p in (0, 1)]
        TTB = [const.tile([128, 1026], bf16, name=f"TTB{p}") for p in (0, 1)]
        TE = [const.tile([128, 1026], bf16, name=f"TE{p}") for p in (0, 1)]
        TDL = [const.tile([128, 1026], bf16, name=f"TDL{p}") for p in (0, 1)]
        for t in TTB + TE + TDL:
            msets.append(nc.gpsimd.memset(t[:, :], 0.0))
        PPW = const.tile([128, NW * 1026], bf16, name="PPW")
        TDW = const.tile([128, NW * 1026], bf16, name="TDW")
        msets.append(nc.gpsimd.memset(PPW[:, :], 0.0))
        msets.append(nc.gpsimd.memset(TDW[:, :], 0.0))

        # accumulators
        ACTACC = accp.tile([128, NQ_ACT * NWIN], fp32, name="actacc")
        ACCBIG = accp.tile([128, NQ * NWIN], fp32, name="accbig")

        def acc(q, gw):
            if q < NQ_ACT:
                return ACTACC[:, q * NWIN + gw:q * NWIN + gw + 1]
            return ACCBIG[:, q * NWIN + gw:q * NWIN + gw + 1]

        # stats scratch
        FIN = const.tile([128, 2 * NW], fp32, name="fin")
        DS = const.tile([128, 2], fp32, name="ds")
        ST = const.tile([128, 16], fp32, name="st")

        # ---- startup observers ----
        DOBS1 = pse.tile([128, 1024], fp32, tag="pse", name="dobs1")
        nc.tensor.matmul(DOBS1[:, 0:1], CB[:, 0:128], CB[:, 0:1],
                         start=True, stop=True)
        DOBS2 = pse.tile([128, 1024], fp32, tag="pse", name="dobs2")
        nc.tensor.matmul(DOBS2[:, 0:1], ONESF, ONESF[:, 0:1],
                         start=True, stop=True)
        DOBS3 = pse.tile([128, 1024], fp32, tag="pse", name="dobs3")
        d3 = nc.tensor.matmul(DOBS3[:, 0:1], CB[:, 0:128],
                              TDW[:, NW * 1026 - 1:NW * 1026],
                              start=True, stop=True)
        OBSA = const.tile([128, 1], bf16, name="obsa")
        oa = nc.scalar.activation(out=OBSA, in_=EPSB, func=ACTF.Copy)
        DVOBS = const.tile([128, 1], fp32, name="dvobs")
        dv = nc.vector.tensor_scalar(out=DVOBS,
                                     in0=TDW[:, NW * 1026 - 1:NW * 1026],
                                     scalar1=1.0, scalar2=None, op0=ALU.mult)
        # scheduler may reorder memsets; pin every observer after ALL of them
        for obs in (d3, oa, dv):
            for m in msets:
                tile.add_dep_helper(obs.ins, m.ins, sync=True,
                                    reason="observe all memsets")

        def conv(ps, groups, srctile):
            for c0 in (0, 512):
                for i, (mat, dx) in enumerate(groups):
                    nc.tensor.matmul(
                        ps[:, c0:c0 + 512], mat,
                        srctile[:, c0 + dx + 1:c0 + dx + 1 + 512],
                        start=(i == 0), stop=(i == len(groups) - 1))

        accs_cur = []

        def dve_acc(src, q, gw):
            j = scr.tile([128, 1024], bf16, tag="jacc", name=f"jacc{q}_{gw}")
            i = nc.vector.tensor_scalar(
                out=j, in0=src, scalar1=1.0, scalar2=0.0, op0=ALU.mult,
                op1=ALU.add, accum_out=acc(q, gw))
            accs_cur.append(i)
            return i

        rd_dve, rd_act, rd_pool = {}, {}, {}
        input_dmas = []
        et_last = cs_prev = muex_prev = et_prev = ph_last = None

        for s in range(SPC):
            inv_ap = ST[:, 8 * s + 6:8 * s + 7]
            nb_ap = ST[:, 8 * s + 7:8 * s + 8]
            if s > 0:
                # ACT observes Pool >= prodPH(prev sample last) so PPW/hn
                # WARs vs prior-sample Pool readers are implied.
                nc.scalar.activation(out=OBSA, in_=ph_last[:, 0:1],
                                     func=ACTF.Copy)
            for wi, (r0, p0, p1) in enumerate(WINDOWS):
                gw = s * NW + wi
                par = gw % 2
                Tt, Tp, Td = TT[par], TP[par], TD[par]
                Ttb, Te, Tdl = TTB[par], TE[par], TDL[par]
                PPs = PPW[:, wi * 1026:(wi + 1) * 1026]
                TDs = TDW[:, wi * 1026:(wi + 1) * 1026]

                # WAR absorber chain: readers of the par buffers from gw-2,
                # grouped per engine; DMAs follow in SP program order.
                last_nop = None
                if gw >= 2:
                    for rdmap in (rd_dve, rd_act, rd_pool):
                        n = nc.sync.nop()
                        tile.add_dep_helper(n.ins, rdmap[gw - 2].ins, sync=True,
                                            reason="absorb reader WAR")
                        last_nop = n
                for dst, src in ((Tt, targ_d), (Tp, pred_d), (Td, dem_d)):
                    d = nc.sync.dma_start(out=dst, in_=src[s, r0:r0 + 128, :])
                    if last_nop is not None:
                        tile.add_dep_helper(d.ins, last_nop.ins, sync=False,
                                            reason="order after absorber")
                        input_dmas.append(d.ins.name)

                accs_prev, accs_cur = accs_cur, []

                # ---- Pool first-touch + raw-input product tiles ----
                pobs = scr.tile([128, 8], bf16, tag="pobs")
                poi = nc.gpsimd.tensor_tensor(out=pobs, in0=Tt[:, 0:8],
                                              in1=Tt[:, 0:8], op=ALU.add)
                if gw >= 1:
                    tile.add_dep_helper(poi.ins, rd_pool[gw - 1].ins,
                                        sync=True,
                                        reason="keep Pool on window cadence")
                prodTP = scr.tile([128, 1024], bf16, tag="prodTP")
                tpi = nc.gpsimd.tensor_tensor(out=prodTP, in0=Tt, in1=Tp,
                                              op=ALU.mult)
                tile.add_dep_helper(tpi.ins, poi.ins, sync=True,
                                    reason="order after lane observer")
                prodDQ = scr.tile([128, 1024], bf16, tag="prodDQ")
                dqi = nc.gpsimd.tensor_tensor(out=prodDQ, in0=Td, in1=Td,
                                              op=ALU.mult)
                tile.add_dep_helper(dqi.ins, tpi.ins, sync=True,
                                    reason="keep Pool on window cadence")
                dve_acc(prodTP, Q_TP, gw)
                dve_acc(prodDQ, Q_DSQ, gw)

                # ---- DVE converts ----
                cvtt = nc.vector.tensor_scalar(
                    out=Ttb[:, 1:1025], in0=Tt, scalar1=1.0, scalar2=None,
                    op0=ALU.mult)
                if et_prev is not None:
                    tile.add_dep_helper(cvtt.ins, et_prev.ins, sync=True,
                                        reason="order cvtt after Et-thr")
                else:
                    tile.add_dep_helper(cvtt.ins, dv.ins, sync=True,
                                        reason="order first cvtt after DVOBS")
                for a in accs_prev:
                    tile.add_dep_helper(cvtt.ins, a.ins, sync=True,
                                        reason="keep accums on window cadence")
                cvtd = nc.vector.tensor_scalar(
                    out=TDs[:, 1:1025], in0=Td, scalar1=1.0, scalar2=0.0,
                    op0=ALU.mult, op1=ALU.add, accum_out=acc(Q_DSUM, gw))
                for a in accs_prev:
                    tile.add_dep_helper(cvtd.ins, a.ins, sync=True,
                                        reason="keep accums on window cadence")
                if muex_prev is not None:
                    tile.add_dep_helper(cvtd.ins, muex_prev.ins, sync=True,
                                        reason="order cvt after PSW read")
                elif gw == 0:
                    tile.add_dep_helper(cvtd.ins, dv.ins, sync=True,
                                        reason="order first cvtd after DVOBS")
                rd_dve[gw] = cvtd

                # ---- PE: laplacian then box ----
                lp = psl.tile([128, 1024], fp32, tag="psl")
                if cs_prev is not None:
                    nc.tensor.matmul(lp[:, 0:1], CB[:, 0:128],
                                     cs_prev[:, 0:1], start=True, stop=True)
                conv(lp, [(IB, -1), (IB, 1), (MLAPB, 0)], TDs)
                bx = pse.tile([128, 1024], fp32, tag="pse")
                conv(bx, [(M111B, -1), (M111B, 0), (M111B, 1), (M9IB, 0)], Ttb)

                # curvature score: sigmoid(10*tanh(0.1*lp)) ~= sigmoid(lp);
                # cs FIRST in ACT order so later PE waits on pp dominate it.
                cs = scr.tile([128, 1024], bf16, tag="cs", bufs=3)
                csi = nc.scalar.activation(out=cs, in_=lp, func=ACTF.Sigmoid)
                cs_prev = cs
                p1i = nc.scalar.activation(out=PPs[:, 1:1025], in_=Tp,
                                           func=ACTF.Sigmoid)
                if gw <= 1:
                    for i2 in (csi, p1i):
                        tile.add_dep_helper(i2.ins, oa.ins, sync=True,
                                            reason="order after ACT observer")
                rd_act[gw] = p1i

                # ---- edge chain (DVE thresholds) ----
                e1 = scr.tile([128, 1024], bf16, tag="bx2")
                nc.vector.tensor_scalar(out=e1, in0=bx, scalar1=1.35,
                                        scalar2=None, op0=ALU.is_gt)
                e2 = scr.tile([128, 1024], bf16, tag="bx3")
                nc.vector.tensor_scalar(out=e2, in0=bx, scalar1=-1.35,
                                        scalar2=None, op0=ALU.is_lt)
                nc.vector.tensor_tensor(out=Te[:, 1:1025], in0=e1, in1=e2,
                                        op=ALU.add)
                dl = pse.tile([128, 1024], fp32, tag="pse")
                conv(dl, [(M111B, -1), (M111B, 0), (M111B, 1)], Te)
                nc.vector.tensor_scalar(out=Tdl[:, 1:1025], in0=dl, scalar1=0.5,
                                        scalar2=None, op0=ALU.is_gt)
                er = pse.tile([128, 1024], fp32, tag="pse")
                conv(er, [(M111B, -1), (M111B, 0), (M111B, 1)], Tdl)
                Et = scr.tile([128, 1024], bf16, tag="Et", bufs=3)
                et_prev = nc.vector.tensor_scalar(
                    out=Et, in0=er, scalar1=8.5, scalar2=None, op0=ALU.is_gt)
                et_last = Et
                prodEP = scr.tile([128, 1024], bf16, tag="prodEP")
                ep = nc.gpsimd.tensor_tensor(out=prodEP, in0=Et, in1=Tp,
                                             op=ALU.mult)
                rd_pool[gw] = ep
                dve_acc(prodEP, Q_EP, gw)

                # ---- sobel d then sobel p ----
                gxd = pss.tile([128, 1024], fp32, tag="pss")
                conv(gxd, [(M121NB, -1), (M121B, 1)], TDs)
                gyd = pss.tile([128, 1024], fp32, tag="pss")
                conv(gyd, [(M101B, -1), (M101B, 1), (M202B, 0)], TDs)
                xxd = scr.tile([128, 1024], bf16, tag="xxd")
                nc.scalar.activation(out=xxd, in_=gxd, func=ACTF.Square)
                yyd = scr.tile([128, 1024], bf16, tag="yyd")
                nc.scalar.activation(out=yyd, in_=gyd, func=ACTF.Square)
                g2d = scr.tile([128, 1024], bf16, tag="g2d", bufs=3)
                nc.gpsimd.tensor_tensor(out=g2d, in0=xxd, in1=yyd, op=ALU.add)
                nc.vector.tensor_reduce(out=acc(Q_MAXD, gw), in_=g2d,
                                        axis=AXL.X, op=ALU.max)
                nc.vector.tensor_reduce(out=acc(Q_MIND, gw), in_=g2d,
                                        axis=AXL.X, op=ALU.min)
                dve_acc(g2d, Q_G2D, gw)
                avd = scr.tile([128, 1024], bf16, tag="avd", bufs=3)
                nc.scalar.activation(out=avd, in_=g2d, func=ACTF.Sqrt,
                                     bias=EPSB, accum_out=acc(Q_SB, gw))

                gxp = pss.tile([128, 1024], fp32, tag="pss")
                conv(gxp, [(M121NB, -1), (M121B, 1)], PPs)
                gyp = pss.tile([128, 1024], fp32, tag="pss")
                conv(gyp, [(M101B, -1), (M101B, 1), (M202B, 0)], PPs)
                xxp = scr.tile([128, 1024], bf16, tag="xxp")
                nc.scalar.activation(out=xxp, in_=gxp, func=ACTF.Square)
                yyp = scr.tile([128, 1024], bf16, tag="yyp")
                nc.scalar.activation(out=yyp, in_=gyp, func=ACTF.Square)
                g2p = scr.tile([128, 1024], bf16, tag="g2p", bufs=3)
                nc.gpsimd.tensor_tensor(out=g2p, in0=xxp, in1=yyp, op=ALU.add)
                nc.vector.tensor_reduce(out=acc(Q_MAXP, gw), in_=g2p,
                                        axis=AXL.X, op=ALU.max)
                nc.vector.tensor_reduce(out=acc(Q_MINP, gw), in_=g2p,
                                        axis=AXL.X, op=ALU.min)
                dve_acc(g2p, Q_G2P, gw)
                avp = scr.tile([128, 1024], bf16, tag="avp", bufs=3)
                nc.scalar.activation(out=avp, in_=g2p, func=ACTF.Sqrt,
                                     bias=EPSB, accum_out=acc(Q_SA, gw))

                # ---- remaining products ----
                prodAB = scr.tile([128, 1024], bf16, tag="prodAB")
                nc.gpsimd.tensor_tensor(out=prodAB, in0=avp, in1=avd,
                                        op=ALU.mult)
                dve_acc(prodAB, Q_AB, gw)
                prodPC = scr.tile([128, 1024], bf16, tag="prodPC")
                nc.gpsimd.tensor_tensor(out=prodPC, in0=PPs[:, 1:1025],
                                        in1=cs, op=ALU.mult)
                dve_acc(prodPC, Q_CURV, gw)

                # ---- softplus: sum(ln(1-pp)) = -sum(softplus(p)) ----
                spj = scr.tile([128, 1024], bf16, tag="spj")
                nc.scalar.activation(out=spj, in_=PPs[:, 1:1025], func=ACTF.Ln,
                                     scale=-1.0, bias=1.0,
                                     accum_out=acc(Q_SP, gw))

            # ---------- per-sample finalize: dem mean/std ----------
            c9 = s * NW
            dsum_cols = ACCBIG[:, Q_DSUM * NWIN + c9:Q_DSUM * NWIN + c9 + NW]
            dsq_cols = ACCBIG[:, Q_DSQ * NWIN + c9:Q_DSQ * NWIN + c9 + NW]
            m1 = FIN[:, 0:NW]
            m2 = FIN[:, NW:2 * NW]
            nc.vector.tensor_tensor(out=m1, in0=dsum_cols, in1=BMASK,
                                    op=ALU.mult)
            nc.vector.tensor_tensor(out=m2, in0=dsq_cols, in1=BMASK,
                                    op=ALU.mult)
            nc.vector.tensor_reduce(out=DS[:, 0:1], in_=m1, axis=AXL.X,
                                    op=ALU.add)
            r2 = nc.vector.tensor_reduce(out=DS[:, 1:2], in_=m2, axis=AXL.X,
                                         op=ALU.add)
            # 1-col absorber so PSW's slot WAR merges into its DVE wait
            DUM = pse.tile([128, 1024], fp32, tag="pse", name=f"dumm{s}")
            nc.tensor.matmul(DUM[:, 0:1], CB[:, 0:128], et_last[:, 0:1],
                             start=True, stop=True)
            PSW = pse.tile([128, 1024], fp32, tag="pse", name=f"psw{s}")
            nc.tensor.matmul(PSW[:, 0:2], ONESF, DS, start=True, stop=True)
            c8 = 8 * s
            mu = ST[:, c8:c8 + 1]
            ex2 = ST[:, c8 + 1:c8 + 2]
            m2t = ST[:, c8 + 2:c8 + 3]
            vr = ST[:, c8 + 3:c8 + 4]
            sd = ST[:, c8 + 4:c8 + 5]
            sde = ST[:, c8 + 5:c8 + 6]
            muex_prev = nc.vector.tensor_scalar(
                out=ST[:, c8:c8 + 2], in0=PSW[:, 0:2],
                scalar1=1.0 / NPIX, scalar2=None, op0=ALU.mult)
            nc.vector.tensor_tensor(out=m2t, in0=mu, in1=mu, op=ALU.mult)
            nc.vector.tensor_tensor(out=vr, in0=ex2, in1=m2t, op=ALU.subtract)
            nc.scalar.activation(out=sd, in_=vr, func=ACTF.Sqrt,
                                 scale=float(NPIX) / (NPIX - 1))
            nc.vector.tensor_scalar(out=sde, in0=sd, scalar1=EPS, scalar2=None,
                                    op0=ALU.add)
            nc.vector.reciprocal(out=inv_ap, in_=sde)
            nc.vector.scalar_tensor_tensor(out=nb_ap, in0=mu, scalar=-1.0,
                                           in1=inv_ap, op0=ALU.mult,
                                           op1=ALU.mult)
            # PE observes DVE >= nb so the next sample's first conv carries
            # only its input wait (PSW-reader WAR becomes implied).
            DUM2 = pse.tile([128, 1024], fp32, tag="pse", name=f"dumm2{s}")
            nc.tensor.matmul(DUM2[:, 0:1], ONESF, nb_ap, start=True, stop=True)

            # ---------- Phase B: height-norm term ----------
            for wi in range(NW):
                gw = s * NW + wi
                PPs = PPW[:, wi * 1026:(wi + 1) * 1026]
                TDs = TDW[:, wi * 1026:(wi + 1) * 1026]
                accs_prev, accs_cur = accs_cur, []
                z = scr.tile([128, 1024], bf16, tag="z")
                zi = nc.vector.tensor_scalar(out=z, in0=TDs[:, 1:1025],
                                             scalar1=inv_ap, scalar2=nb_ap,
                                             op0=ALU.mult, op1=ALU.add)
                for a in accs_prev:
                    tile.add_dep_helper(zi.ins, a.ins, sync=True,
                                        reason="keep accums on window cadence")
                z2 = scr.tile([128, 1024], bf16, tag="z2")
                nc.vector.tensor_tensor(out=z2, in0=z, in1=z, op=ALU.mult)
                if wi >= 1:
                    # ACT observes Pool >= prodPH(wi-1) so hn's ring WAR is
                    # implied.
                    nc.scalar.activation(out=OBSA, in_=ph_last[:, 0:1],
                                         func=ACTF.Copy)
                hn = scr.tile([128, 1024], bf16, tag="hn", bufs=3)
                nc.scalar.activation(out=hn, in_=z2, func=ACTF.Exp, scale=-0.5)
                pobsB = scr.tile([128, 8], bf16, tag="pobs")
                nc.gpsimd.tensor_tensor(out=pobsB, in0=z2[:, 0:8],
                                        in1=z2[:, 0:8], op=ALU.add)
                prodPH = scr.tile([128, 1024], bf16, tag="prodPH")
                nc.gpsimd.tensor_tensor(out=prodPH, in0=PPs[:, 1:1025],
                                        in1=hn, op=ALU.mult)
                ph_last = prodPH
                dve_acc(prodPH, Q_HGT, gw)

        # ---- final: mirror ACT accumulators into ACCBIG, store ----
        nc.vector.tensor_scalar(out=ACCBIG[:, 0:NQ_ACT * NWIN], in0=ACTACC,
                                scalar1=1.0, scalar2=None, op0=ALU.mult)
        follow = set(os.environ.get("KDBG_FOLLOW2", "").split(",")) - {""}
        if follow:
            for blk in nc.m.functions[0].blocks:
                for ins in blk.instructions:
                    if ins.name in follow:
                        tile.tile_follow(ins, log_all_deps=True)
        nc.sync.dma_start(out=out_d[:, :], in_=ACCBIG[:, :])
        ctx.close()
    nc._input_dma_names = set(input_dmas)

    # ---- sync-wait minimization (walrus allows ONE wait/instruction) ----
    ENG_SEM = {"PE": "PE", "DVE": "DVE", "Activation": "Activation",
               "Pool": "Pool", "SP": "SP_sequencer"}
    observed = {}
    nonmono = set()
    for blk in nc.m.functions[0].blocks:
        for ins in blk.instructions:
            if ins.sync_info is None:
                continue
            for u in ins.sync_info.on_update:
                um = str(u.update_mode)
                if "sub" in um or "dec" in um:
                    nonmono.add(u.ant_name)

    def eng_of(ins):
        e = getattr(ins, "engine", None)
        return str(e).split(".")[-1] if e is not None else "SP"

    # Input-load DMAs are fully gated by their absorber-nop chain; their
    # residual waits are redundant.
    for blk in nc.m.functions[0].blocks:
        for ins in blk.instructions:
            if ins.name in nc._input_dma_names and ins.sync_info is not None:
                ins.sync_info.on_wait = []

    dbg = os.environ.get("KDBG_SYNC")
    for blk in nc.m.functions[0].blocks:
        for ins in blk.instructions:
            si = ins.sync_info
            if si is None:
                continue
            eng = eng_of(ins)
            ws = list(si.on_wait)
            if not ws:
                continue
            kept = []
            for w in ws:
                if w.ant_name not in nonmono and \
                        str(w.wait_mode) == "sem-ge-imm" and \
                        observed.get((eng, w.ant_name), -1) >= w.wait_value:
                    continue
                kept.append(w)
            if len(kept) > 1:
                self_sem = ENG_SEM.get(eng, "zz")
                non_self = [w for w in kept
                            if not w.ant_name.startswith(self_sem)]
                if non_self:
                    kept = non_self
            if len(kept) > 1 and type(ins).__name__ == "InstMatmult":
                nonpe = [w for w in kept if not w.ant_name.startswith("PE")]
                kept = nonpe if nonpe else kept[:1]
            si.on_wait = kept
            for w in kept:
                if w.ant_name in nonmono:
                    continue
                k = (eng, w.ant_name)
                observed[k] = max(observed.get(k, -1), w.wait_value)

    for blk in nc.m.functions[0].blocks:
        for ins in blk.instructions:
            si = ins.sync_info
            if si is None or len(si.on_wait) <= 1:
                continue
            if type(ins).__name__ != "InstDrain":
                if dbg:
                    print(f"KDBG multiwait {type(ins).__name__} "
                          f"{eng_of(ins)} {ins.name}: "
                          f"{[(w.ant_name, w.wait_value) for w in si.on_wait]}")
                nonpe = [w for w in si.on_wait
                         if not w.ant_name.startswith("PE")]
                si.on_wait = nonpe if nonpe else si.on_wait[:1]

    all_dmas = [ins for blk in nc.m.functions[0].blocks
                for ins in blk.instructions
                if type(ins).__name__ == "InstDMACopy"]
    if all_dmas:
        fin = all_dmas[-1]
        if fin.sync_info and len(fin.sync_info.on_wait) > 1:
            eng = [w for w in fin.sync_info.on_wait
                   if not w.ant_name.startswith(("DMAHW", "DMASW"))]
            if eng:
                fin.sync_info.on_wait = eng

    out_dmas = all_dmas[-1:]
    keep_lanes = set()
    for ins in out_dmas:
        for u in (ins.sync_info.on_update if ins.sync_info else []):
            if u.ant_name.startswith(("DMAHW", "DMASW")):
                keep_lanes.add(u.ant_name)
    for blk in nc.m.functions[0].blocks:
        for ins in blk.instructions:
            if type(ins).__name__ == "InstDrain" and ins.sync_info and \
                    len(ins.sync_info.on_wait) > 1:
                lane_ws = [w for w in ins.sync_info.on_wait
                           if w.ant_name in keep_lanes]
                if lane_ws:
                    ins.sync_info.on_wait = lane_ws
    for blk in nc.m.functions[0].blocks:
        bad = [i for i, ins in enumerate(blk.instructions)
               if type(ins).__name__ == "InstISA"]
        if bad:
            keep = [ins for ins in blk.instructions
                    if type(ins).__name__ != "InstISA"]
            try:
                blk.instructions = keep
            except Exception:
                for i in reversed(bad):
                    del blk.instructions[i]
    _NC_CACHE["nc"] = nc
    return nc


def _combine(outs):
    """outs: list of [128, NQ*NWIN] f32 arrays (one per core). float64 combine."""
    A = np.stack([o.reshape(128, NQ, NWIN).astype(np.float64) for o in outs])
    m = np.concatenate([_band_mask9()] * SPC, axis=1)[None, :, None, :]
    sums = (A * m).sum(axis=(0, 1, 3))
    s_sp = -sums[Q_SP]  # device accumulates ln(1-pp) = -softplus(p)
    s_tp, s_ep = sums[Q_TP], sums[Q_EP]
    s_g2p, s_g2d = sums[Q_G2P], sums[Q_G2D]
    s_a, s_b = sums[Q_SA], sums[Q_SB]
    s_ab, s_curv, s_hgt = sums[Q_AB], sums[Q_CURV], sums[Q_HGT]
    mm = m[:, :, 0, :]
    min_g2p = np.where(mm > 0, A[:, :, Q_MINP, :], FBIG).min()
    max_g2p = np.where(mm > 0, A[:, :, Q_MAXP, :], -FBIG).max()
    min_g2d = np.where(mm > 0, A[:, :, Q_MIND, :], FBIG).min()
    max_g2d = np.where(mm > 0, A[:, :, Q_MAXD, :], -FBIG).max()

    bce1 = (s_sp - s_tp) / NTOT
    bce2 = (s_sp - s_ep) / NTOT

    e_a2 = s_g2p / NTOT + EPS
    e_b2 = s_g2d / NTOT + EPS
    amin, amax = np.sqrt(min_g2p + EPS), np.sqrt(max_g2p + EPS)
    bmin, bmax = np.sqrt(min_g2d + EPS), np.sqrt(max_g2d + EPS)

    def scale_off(lo, hi):
        if hi > lo:
            sc = 1.0 / (hi - lo + EPS)
            return sc, lo * sc
        return 1.0, 0.0

    sa, oa = scale_off(amin, amax)
    sb, ob = scale_off(bmin, bmax)
    cc = oa - ob
    e_a, e_b, e_ab = s_a / NTOT, s_b / NTOT, s_ab / NTOT
    grad_cons = (sa * sa * e_a2 + sb * sb * e_b2 + cc * cc
                 - 2.0 * sa * sb * e_ab - 2.0 * cc * sa * e_a
                 + 2.0 * cc * sb * e_b)

    height_cons = -s_hgt / NTOT
    curv_cons = -s_curv / NTOT
    geo = grad_cons + 0.5 * height_cons + 0.3 * curv_cons
    total = 0.8 * bce1 + 0.1 * bce2 + 0.1 * geo
    return np.float32(total)


_CONSTS = {}
_ONES128 = np.ones((128, 128), np.float32)


def kernel(pred, target, dem, _profile=False):
    from concourse.bass_utils import run_bass_kernel_spmd

    if "c" not in _CONSTS:
        _CONSTS["c"] = _build_consts()
        _CONSTS["bm"] = _band_mask9()
    cbf16 = _CONSTS["c"]
    bmask = _CONSTS["bm"]
    nc = _build_nc()

    p = np.ascontiguousarray(pred.reshape(B, H, W), dtype=np.float32)
    t = np.ascontiguousarray(target.reshape(B, H, W), dtype=np.float32)
    d = np.ascontiguousarray(dem.reshape(B, H, W), dtype=np.float32)
    in_maps = []
    for c in range(NCORES):
        sl = slice(c * SPC, (c + 1) * SPC)
        in_maps.append({
            "pred": p[sl], "target": t[sl], "dem": d[sl],
            "cbf16": cbf16, "onesf": _ONES128, "bmask": bmask,
        })
    res = run_bass_kernel_spmd(nc, in_maps, core_ids=list(range(NCORES)),
                               trace=_profile)
    outs = [m["out"] for m in res.results]
    total = _combine(outs)
    if _profile:
        return total, res
    return total


# revision 57
# speedup vs baseline: 1.7327x; 1.5427x over previous
"""Trainium2 Bass kernel for nn_BalancedLoss (composite segmentation loss).

Data-parallel over 8 NeuronCores (2 samples each). Each core emits a
[128, NQ*NWIN] tensor of per-window partial reductions; the host combines
them in float64 (global min/max normalization handled via moment algebra).

v3 restructure vs baseline (915us):
  - No dem-stats prepass: dem sum/sumsq accumulate during the main windows,
    per-sample mean/std finalized on device, and the height-norm term runs
    as a second pass over SBUF-resident bf16 sigmoid(pred)/dem (no extra HBM
    traffic).
  - Engine rebalance within ISA limits: Pool (GPSIMD) takes product tiles /
    g2 adds (tensor_tensor, SBUF-only); DVE does thresholds, reductions and
    cheap 4x-mode accumulate-sums of the Pool product tiles; ACT does
    sigmoid/ln/sqrt/square ordered to minimize ACT_TABLE_LOADs, with
    softplus folded into -ln(1-sigmoid(p)) to reuse the sigmoid tile.
  - Whole-tile DMAs (one HWDGE lane per tile) so full-width consumers carry
    a single wait; the graph is engineered so every instruction needs at
    most ONE hardware sync-wait (walrus limit).
"""

import os
import numpy as np
from contextlib import ExitStack

B, H, W = 16, 1024, 1024
NCORES = 8
SPC = B // NCORES  # samples per core
EPS = 1e-8
NPIX = H * W
NTOT = B * NPIX

# window row-starts and valid partition bands [p0, p1)
WINDOWS = [(0, 0, 125)] + [(122 * w, 3, 125) for w in range(1, 8)] + [(896, 83, 128)]
NW = len(WINDOWS)
NWIN = SPC * NW

# quantity indices: [0..3) ACT-written, [3..16) DVE-written
Q_SP, Q_SA, Q_SB = 0, 1, 2  # Q_SP holds sum(ln(1-pp)) = -sum(softplus(p))
Q_G2P, Q_G2D, Q_MAXP, Q_MINP, Q_MAXD, Q_MIND = 3, 4, 5, 6, 7, 8
Q_TP, Q_EP, Q_DSQ, Q_AB, Q_CURV, Q_HGT, Q_DSUM = 9, 10, 11, 12, 13, 14, 15
NQ_ACT = 3
NQ = 16

FBIG = 3.0e38


def _tridiag(a, b, c, dtype):
    # out[p] = a*x[p-1] + b*x[p] + c*x[p+1] for matmul out = lhsT.T @ x
    M = np.zeros((128, 128), dtype=np.float64)
    idx = np.arange(128)
    M[idx, idx] = b
    M[idx[:-1], idx[1:]] = a  # row k=p-1, col p
    M[idx[1:], idx[:-1]] = c  # row k=p+1, col p
    return M.astype(dtype)


def _build_consts():
    import ml_dtypes
    bf16 = ml_dtypes.bfloat16
    mats = [
        _tridiag(1, 1, 1, bf16),                 # 0 M111
        _tridiag(1, 2, 1, bf16),                 # 1 M121
        _tridiag(-1, -2, -1, bf16),              # 2 -M121
        _tridiag(-1, 0, 1, bf16),                # 3 Mm101
        _tridiag(-2, 0, 2, bf16),                # 4 Mm202
        _tridiag(0, -9, 0, bf16),                # 5 -9I
        _tridiag(0, 1, 0, bf16),                 # 6 I
        _tridiag(1, -4, 1, bf16),                # 7 M1m41
    ]
    return np.concatenate(mats, axis=1)  # [128, 8*128]


def _band_mask9():
    m = np.zeros((128, NW), np.float32)
    for wi, (r0, p0, p1) in enumerate(WINDOWS):
        m[p0:p1, wi] = 1.0
    return m


_NC_CACHE = {}


def _build_nc():
    if "nc" in _NC_CACHE:
        return _NC_CACHE["nc"]
    import concourse.bass as bass
    import concourse.tile as tile
    from concourse import mybir

    fp32 = mybir.dt.float32
    bf16 = mybir.dt.bfloat16
    ALU = mybir.AluOpType
    ACTF = mybir.ActivationFunctionType
    AXL = mybir.AxisListType

    nc = bass.Bass("TRN2", target_bir_lowering=False)
    pred_d = nc.declare_dram_parameter("pred", [SPC, H, W], fp32, isOutput=False)
    targ_d = nc.declare_dram_parameter("target", [SPC, H, W], fp32, isOutput=False)
    dem_d = nc.declare_dram_parameter("dem", [SPC, H, W], fp32, isOutput=False)
    cbf16_d = nc.declare_dram_parameter("cbf16", [128, 8 * 128], bf16,
                                        isOutput=False)
    ones_d = nc.declare_dram_parameter("onesf", [128, 128], fp32, isOutput=False)
    bmask_d = nc.declare_dram_parameter("bmask", [128, NW], fp32, isOutput=False)
    out_d = nc.declare_dram_parameter("out", [128, NQ * NWIN], fp32, isOutput=True)

    with tile.TileContext(nc) as tc:
        ctx = ExitStack()
        const = ctx.enter_context(tc.tile_pool(name="const", bufs=1))
        accp = ctx.enter_context(tc.tile_pool(name="accp", bufs=1))
        scr = ctx.enter_context(tc.tile_pool(name="scr", bufs=2))
        pse = ctx.enter_context(tc.tile_pool(name="pse", bufs=1, space="PSUM"))
        psl = ctx.enter_context(tc.tile_pool(name="psl", bufs=1, space="PSUM"))
        pss = ctx.enter_context(tc.tile_pool(name="pss", bufs=2, space="PSUM"))

        # ---- consts (3 DMAs -> 3 HWDGE lanes) ----
        CB = const.tile([128, 8 * 128], bf16)
        nc.sync.dma_start(out=CB, in_=cbf16_d[:, :])
        ONESF = const.tile([128, 128], fp32)
        nc.sync.dma_start(out=ONESF, in_=ones_d[:, :])
        BMASK = const.tile([128, NW], fp32)
        nc.sync.dma_start(out=BMASK, in_=bmask_d[:, :])

        EPSB = const.tile([128, 1], fp32)
        msets = [nc.gpsimd.memset(EPSB, EPS)]

        def mb(i):
            return CB[:, i * 128:(i + 1) * 128]

        M111B, M121B, M121NB, M101B, M202B, M9IB, IB, MLAPB = (
            mb(0), mb(1), mb(2), mb(3), mb(4), mb(5), mb(6), mb(7))

        # ---- persistent tiles ----
        TT = [const.tile([128, 1024], fp32, name=f"TT{p}") for p in (0, 1)]
        TP = [const.tile([128, 1024], fp32, name=f"TP{p}") for p in (0, 1)]
        TD = [const.tile([128, 1024], fp32, name=f"TD{p}") for p in (0, 1)]
        TTB = [const.tile([128, 1026], bf16, name=f"TTB{p}") for p in (0, 1)]
        TE = [const.tile([128, 1026], bf16, name=f"TE{p}") for p in (0, 1)]
        TDL = [const.tile([128, 1026], bf16, name=f"TDL{p}") for p in (0, 1)]
        for t in TTB + TE + TDL:
            msets.append(nc.gpsimd.memset(t[:, :], 0.0))
        PPW = const.tile([128, NW * 1026], bf16, name="PPW")
        TDW = const.tile([128, NW * 1026], bf16, name="TDW")
        msets.append(nc.gpsimd.memset(PPW[:, :], 0.0))
        msets.append(nc.gpsimd.memset(TDW[:, :], 0.0))

        # accumulators
        ACTACC = accp.tile([128, NQ_ACT * NWIN], fp32, name="actacc")
        ACCBIG = accp.tile([128, NQ * NWIN], fp32, name="accbig")

        def acc(q, gw):
            if q < NQ_ACT:
                return ACTACC[:, q * NWIN + gw:q * NWIN + gw + 1]
            return ACCBIG[:, q * NWIN + gw:q * NWIN + gw + 1]

        # stats scratch
        FIN = const.tile([128, 2 * NW], fp32, name="fin")
        DS = const.tile([128, 2], fp32, name="ds")
        ST = const.tile([128, 16], fp32, name="st")

        # ---- startup observers ----
        DOBS1 = pse.tile([128, 1024], fp32, tag="pse", name="dobs1")
        nc.tensor.matmul(DOBS1[:, 0:1], CB[:, 0:128], CB[:, 0:1],
                         start=True, stop=True)
        DOBS2 = pse.tile([128, 1024], fp32, tag="pse", name="dobs2")
        nc.tensor.matmul(DOBS2[:, 0:1], ONESF, ONESF[:, 0:1],
                         start=True, stop=True)
        DOBS3 = pse.tile([128, 1024], fp32, tag="pse", name="dobs3")
        d3 = nc.tensor.matmul(DOBS3[:, 0:1], CB[:, 0:128],
                              TDW[:, NW * 1026 - 1:NW * 1026],
                              start=True, stop=True)
        OBSA = const.tile([128, 1], bf16, name="obsa")
        oa = nc.scalar.activation(out=OBSA, in_=EPSB, func=ACTF.Copy)
        DVOBS = const.tile([128, 1], fp32, name="dvobs")
        dv = nc.vector.tensor_scalar(out=DVOBS,
                                     in0=TDW[:, NW * 1026 - 1:NW * 1026],
                                     scalar1=1.0, scalar2=None, op0=ALU.mult)
        # scheduler may reorder memsets; pin every observer after ALL of them
        for obs in (d3, oa, dv):
            for m in msets:
                tile.add_dep_helper(obs.ins, m.ins, sync=True,
                                    reason="observe all memsets")

        def conv(ps, groups, srctile):
            for c0 in (0, 512):
                for i, (mat, dx) in enumerate(groups):
                    nc.tensor.matmul(
                        ps[:, c0:c0 + 512], mat,
                        srctile[:, c0 + dx + 1:c0 + dx + 1 + 512],
                        start=(i == 0), stop=(i == len(groups) - 1))

        accs_cur = []

        def stt_acc(a, b, q, gw, op1=None):
            j = scr.tile([128, 1024], bf16, tag="jacc", name=f"jacc{q}_{gw}")
            i = nc.vector.scalar_tensor_tensor(
                out=j, in0=a, scalar=1.0, in1=b, op0=ALU.mult,
                op1=op1 or ALU.mult, accum_out=acc(q, gw))
            accs_cur.append(i)
            return i

        rd_dve, rd_act = {}, {}
        input_dmas = []
        et_last = cs_prev = muex_prev = et_prev = None

        for s in range(SPC):
            inv_ap = ST[:, 8 * s + 6:8 * s + 7]
            nb_ap = ST[:, 8 * s + 7:8 * s + 8]
            if s > 0:
                # ACT observes DVE >= s5(prev sample last) so PPW/hn WARs
                # vs prior-sample DVE readers are implied.
                oa = nc.scalar.activation(out=OBSA,
                                          in_=acc(Q_HGT, s * NW - 1),
                                          func=ACTF.Copy)
            for wi, (r0, p0, p1) in enumerate(WINDOWS):
                gw = s * NW + wi
                par = gw % 2
                Tt, Tp, Td = TT[par], TP[par], TD[par]
                Ttb, Te, Tdl = TTB[par], TE[par], TDL[par]
                PPs = PPW[:, wi * 1026:(wi + 1) * 1026]
                TDs = TDW[:, wi * 1026:(wi + 1) * 1026]

                # WAR absorber chain: readers of the par buffers from gw-2,
                # grouped per engine; DMAs follow in SP program order.
                last_nop = None
                if gw >= 2:
                    n = nc.sync.nop()
                    for r in rd_dve[gw - 2]:
                        tile.add_dep_helper(n.ins, r.ins, sync=True,
                                            reason="absorb reader WAR")
                    last_nop = nc.sync.nop()
                    tile.add_dep_helper(last_nop.ins, rd_act[gw - 2].ins,
                                        sync=True, reason="absorb reader WAR")
                for dst, src in ((Tt, targ_d), (Tp, pred_d), (Td, dem_d)):
                    d = nc.sync.dma_start(out=dst, in_=src[s, r0:r0 + 128, :])
                    if last_nop is not None:
                        tile.add_dep_helper(d.ins, last_nop.ins, sync=False,
                                            reason="order after absorber")
                        input_dmas.append(d.ins.name)

                accs_prev, accs_cur = accs_cur, []

                # ---- DVE converts ----
                cvtt = nc.vector.tensor_scalar(
                    out=Ttb[:, 1:1025], in0=Tt, scalar1=1.0, scalar2=None,
                    op0=ALU.mult)
                if et_prev is not None:
                    tile.add_dep_helper(cvtt.ins, et_prev.ins, sync=True,
                                        reason="order cvtt after Et-thr")
                else:
                    tile.add_dep_helper(cvtt.ins, dv.ins, sync=True,
                                        reason="order first cvtt after DVOBS")
                for a in accs_prev:
                    tile.add_dep_helper(cvtt.ins, a.ins, sync=True,
                                        reason="keep accums on window cadence")
                cvtd = nc.vector.tensor_scalar(
                    out=TDs[:, 1:1025], in0=Td, scalar1=1.0, scalar2=0.0,
                    op0=ALU.mult, op1=ALU.add, accum_out=acc(Q_DSUM, gw))
                for a in accs_prev:
                    tile.add_dep_helper(cvtd.ins, a.ins, sync=True,
                                        reason="keep accums on window cadence")
                if muex_prev is not None:
                    tile.add_dep_helper(cvtd.ins, muex_prev.ins, sync=True,
                                        reason="order cvt after PSW read")
                elif gw == 0:
                    tile.add_dep_helper(cvtd.ins, dv.ins, sync=True,
                                        reason="order first cvtd after DVOBS")
                s1i = stt_acc(Tt, Tp, Q_TP, gw)
                tile.add_dep_helper(s1i.ins, cvtt.ins, sync=True,
                                    reason="order after Tt first-touch")
                dqi = stt_acc(Td, Td, Q_DSQ, gw)
                tile.add_dep_helper(dqi.ins, cvtd.ins, sync=True,
                                    reason="order after Td first-touch")
                rd_dve[gw] = [cvtt, cvtd, s1i, dqi]

                # ---- PE: laplacian then box ----
                lp = psl.tile([128, 1024], fp32, tag="psl")
                if cs_prev is not None:
                    nc.tensor.matmul(lp[:, 0:1], CB[:, 0:128],
                                     cs_prev[:, 0:1], start=True, stop=True)
                conv(lp, [(IB, -1), (IB, 1), (MLAPB, 0)], TDs)
                bx = pse.tile([128, 1024], fp32, tag="pse")
                conv(bx, [(M111B, -1), (M111B, 0), (M111B, 1), (M9IB, 0)], Ttb)

                # curvature score: sigmoid(10*tanh(0.1*lp)) ~= sigmoid(lp);
                # cs FIRST in ACT order so later PE waits on pp dominate it.
                cs = scr.tile([128, 1024], bf16, tag="cs", bufs=3)
                csi = nc.scalar.activation(out=cs, in_=lp, func=ACTF.Sigmoid)
                cs_prev = cs
                p1i = nc.scalar.activation(out=PPs[:, 1:1025], in_=Tp,
                                           func=ACTF.Sigmoid)
                if gw <= 1 or wi <= 1:
                    for i2 in (csi, p1i):
                        tile.add_dep_helper(i2.ins, oa.ins, sync=True,
                                            reason="order after ACT observer")
                rd_act[gw] = p1i

                # ---- edge chain (DVE thresholds) ----
                xxb = scr.tile([128, 1024], bf16, tag="bx2")
                nc.scalar.activation(out=xxb, in_=bx, func=ACTF.Square)
                nc.vector.tensor_scalar(out=Te[:, 1:1025], in0=xxb,
                                        scalar1=1.8225, scalar2=None,
                                        op0=ALU.is_gt)
                dl = pse.tile([128, 1024], fp32, tag="pse")
                # 1-col absorber: PE observes ACT >= Square(bx) so dl's slot
                # WAR merges away; dl then waits only on Te (DVE).
                nc.tensor.matmul(dl[:, 0:1], CB[:, 0:128], xxb[:, 0:1],
                                 start=True, stop=True)
                conv(dl, [(M111B, -1), (M111B, 0), (M111B, 1)], Te)
                nc.vector.tensor_scalar(out=Tdl[:, 1:1025], in0=dl, scalar1=0.5,
                                        scalar2=None, op0=ALU.is_gt)
                er = pse.tile([128, 1024], fp32, tag="pse")
                conv(er, [(M111B, -1), (M111B, 0), (M111B, 1)], Tdl)
                Et = scr.tile([128, 1024], bf16, tag="Et", bufs=3)
                et_prev = nc.vector.tensor_scalar(
                    out=Et, in0=er, scalar1=8.5, scalar2=None, op0=ALU.is_gt)
                et_last = Et
                s2i = stt_acc(Et, Tp, Q_EP, gw)
                rd_dve[gw].append(s2i)

                # ---- sobel d then sobel p ----
                gxd = pss.tile([128, 1024], fp32, tag="pss")
                conv(gxd, [(M121NB, -1), (M121B, 1)], TDs)
                gyd = pss.tile([128, 1024], fp32, tag="pss")
                conv(gyd, [(M101B, -1), (M101B, 1), (M202B, 0)], TDs)
                xxd = scr.tile([128, 1024], bf16, tag="xxd")
                nc.scalar.activation(out=xxd, in_=gxd, func=ACTF.Square)
                yyd = scr.tile([128, 1024], bf16, tag="yyd")
                nc.scalar.activation(out=yyd, in_=gyd, func=ACTF.Square)
                g2d = scr.tile([128, 1024], bf16, tag="g2d", bufs=3)
                gi = nc.vector.scalar_tensor_tensor(
                    out=g2d, in0=xxd, scalar=1.0, in1=yyd, op0=ALU.mult,
                    op1=ALU.add, accum_out=acc(Q_G2D, gw))
                accs_cur.append(gi)
                nc.vector.tensor_reduce(out=acc(Q_MAXD, gw), in_=g2d,
                                        axis=AXL.X, op=ALU.max)
                nc.vector.tensor_reduce(out=acc(Q_MIND, gw), in_=g2d,
                                        axis=AXL.X, op=ALU.min)
                avd = scr.tile([128, 1024], bf16, tag="avd", bufs=3)
                nc.scalar.activation(out=avd, in_=g2d, func=ACTF.Sqrt,
                                     bias=EPSB, accum_out=acc(Q_SB, gw))

                gxp = pss.tile([128, 1024], fp32, tag="pss")
                conv(gxp, [(M121NB, -1), (M121B, 1)], PPs)
                gyp = pss.tile([128, 1024], fp32, tag="pss")
                conv(gyp, [(M101B, -1), (M101B, 1), (M202B, 0)], PPs)
                xxp = scr.tile([128, 1024], bf16, tag="xxp")
                nc.scalar.activation(out=xxp, in_=gxp, func=ACTF.Square)
                yyp = scr.tile([128, 1024], bf16, tag="yyp")
                nc.scalar.activation(out=yyp, in_=gyp, func=ACTF.Square)
                g2p = scr.tile([128, 1024], bf16, tag="g2p", bufs=3)
                gi = nc.vector.scalar_tensor_tensor(
                    out=g2p, in0=xxp, scalar=1.0, in1=yyp, op0=ALU.mult,
                    op1=ALU.add, accum_out=acc(Q_G2P, gw))
                accs_cur.append(gi)
                nc.vector.tensor_reduce(out=acc(Q_MAXP, gw), in_=g2p,
                                        axis=AXL.X, op=ALU.max)
                nc.vector.tensor_reduce(out=acc(Q_MINP, gw), in_=g2p,
                                        axis=AXL.X, op=ALU.min)
                avp = scr.tile([128, 1024], bf16, tag="avp", bufs=3)
                nc.scalar.activation(out=avp, in_=g2p, func=ACTF.Sqrt,
                                     bias=EPSB, accum_out=acc(Q_SA, gw))

                # ---- remaining products ----
                stt_acc(avp, avd, Q_AB, gw)
                stt_acc(PPs[:, 1:1025], cs, Q_CURV, gw)

                # ---- softplus: sum(ln(1-pp)) = -sum(softplus(p)) ----
                spj = scr.tile([128, 1024], bf16, tag="spj")
                nc.scalar.activation(out=spj, in_=PPs[:, 1:1025], func=ACTF.Ln,
                                     scale=-1.0, bias=1.0,
                                     accum_out=acc(Q_SP, gw))

            # ---------- per-sample finalize: dem mean/std ----------
            c9 = s * NW
            dsum_cols = ACCBIG[:, Q_DSUM * NWIN + c9:Q_DSUM * NWIN + c9 + NW]
            dsq_cols = ACCBIG[:, Q_DSQ * NWIN + c9:Q_DSQ * NWIN + c9 + NW]
            m1 = FIN[:, 0:NW]
            m2 = FIN[:, NW:2 * NW]
            nc.vector.tensor_tensor(out=m1, in0=dsum_cols, in1=BMASK,
                                    op=ALU.mult)
            nc.vector.tensor_tensor(out=m2, in0=dsq_cols, in1=BMASK,
                                    op=ALU.mult)
            nc.vector.tensor_reduce(out=DS[:, 0:1], in_=m1, axis=AXL.X,
                                    op=ALU.add)
            r2 = nc.vector.tensor_reduce(out=DS[:, 1:2], in_=m2, axis=AXL.X,
                                         op=ALU.add)
            # 1-col absorber so PSW's slot WAR merges into its DVE wait
            DUM = pse.tile([128, 1024], fp32, tag="pse", name=f"dumm{s}")
            nc.tensor.matmul(DUM[:, 0:1], CB[:, 0:128], et_last[:, 0:1],
                             start=True, stop=True)
            PSW = pse.tile([128, 1024], fp32, tag="pse", name=f"psw{s}")
            nc.tensor.matmul(PSW[:, 0:2], ONESF, DS, start=True, stop=True)
            c8 = 8 * s
            mu = ST[:, c8:c8 + 1]
            ex2 = ST[:, c8 + 1:c8 + 2]
            m2t = ST[:, c8 + 2:c8 + 3]
            vr = ST[:, c8 + 3:c8 + 4]
            sd = ST[:, c8 + 4:c8 + 5]
            sde = ST[:, c8 + 5:c8 + 6]
            muex_prev = nc.vector.tensor_scalar(
                out=ST[:, c8:c8 + 2], in0=PSW[:, 0:2],
                scalar1=1.0 / NPIX, scalar2=None, op0=ALU.mult)
            nc.vector.tensor_tensor(out=m2t, in0=mu, in1=mu, op=ALU.mult)
            nc.vector.tensor_tensor(out=vr, in0=ex2, in1=m2t, op=ALU.subtract)
            nc.scalar.activation(out=sd, in_=vr, func=ACTF.Sqrt,
                                 scale=float(NPIX) / (NPIX - 1))
            nc.vector.tensor_scalar(out=sde, in0=sd, scalar1=EPS, scalar2=None,
                                    op0=ALU.add)
            nc.vector.reciprocal(out=inv_ap, in_=sde)
            nc.vector.scalar_tensor_tensor(out=nb_ap, in0=mu, scalar=-1.0,
                                           in1=inv_ap, op0=ALU.mult,
                                           op1=ALU.mult)
            # PE observes DVE >= nb so the next sample's first conv carries
            # only its input wait (PSW-reader WAR becomes implied).
            DUM2 = pse.tile([128, 1024], fp32, tag="pse", name=f"dumm2{s}")
            nc.tensor.matmul(DUM2[:, 0:1], ONESF, nb_ap, start=True, stop=True)

            # ---------- Phase B: height-norm term ----------
            for wi in range(NW):
                gw = s * NW + wi
                PPs = PPW[:, wi * 1026:(wi + 1) * 1026]
                TDs = TDW[:, wi * 1026:(wi + 1) * 1026]
                accs_prev, accs_cur = accs_cur, []
                z = scr.tile([128, 1024], bf16, tag="z")
                zi = nc.vector.tensor_scalar(out=z, in0=TDs[:, 1:1025],
                                             scalar1=inv_ap, scalar2=nb_ap,
                                             op0=ALU.mult, op1=ALU.add)
                for a in accs_prev:
                    tile.add_dep_helper(zi.ins, a.ins, sync=True,
                                        reason="keep accums on window cadence")
                z2 = scr.tile([128, 1024], bf16, tag="z2")
                nc.vector.tensor_tensor(out=z2, in0=z, in1=z, op=ALU.mult)
                hn = scr.tile([128, 1024], bf16, tag="hn", bufs=3)
                nc.scalar.activation(out=hn, in_=z2, func=ACTF.Exp, scale=-0.5)
                stt_acc(PPs[:, 1:1025], hn, Q_HGT, gw)

        # ---- final: mirror ACT accumulators into ACCBIG, store ----
        nc.vector.tensor_scalar(out=ACCBIG[:, 0:NQ_ACT * NWIN], in0=ACTACC,
                                scalar1=1.0, scalar2=None, op0=ALU.mult)
        follow = set(os.environ.get("KDBG_FOLLOW2", "").split(",")) - {""}
        if follow:
            for blk in nc.m.functions[0].blocks:
                for ins in blk.instructions:
                    if ins.name in follow:
                        tile.tile_follow(ins, log_all_deps=True)
        nc.sync.dma_start(out=out_d[:, :], in_=ACCBIG[:, :])
        ctx.close()
    nc._input_dma_names = set(input_dmas)

    # ---- sync-wait minimization (walrus allows ONE wait/instruction) ----
    ENG_SEM = {"PE": "PE", "DVE": "DVE", "Activation": "Activation",
               "Pool": "Pool", "SP": "SP_sequencer"}
    observed = {}
    nonmono = set()
    for blk in nc.m.functions[0].blocks:
        for ins in blk.instructions:
            if ins.sync_info is None:
                continue
            for u in ins.sync_info.on_update:
                um = str(u.update_mode)
                if "sub" in um or "dec" in um:
                    nonmono.add(u.ant_name)

    def eng_of(ins):
        e = getattr(ins, "engine", None)
        return str(e).split(".")[-1] if e is not None else "SP"

    # Input-load DMAs are fully gated by their absorber-nop chain; their
    # residual waits are redundant.
    for blk in nc.m.functions[0].blocks:
        for ins in blk.instructions:
            if ins.name in nc._input_dma_names and ins.sync_info is not None:
                ins.sync_info.on_wait = []

    dbg = os.environ.get("KDBG_SYNC")
    for blk in nc.m.functions[0].blocks:
        for ins in blk.instructions:
            si = ins.sync_info
            if si is None:
                continue
            eng = eng_of(ins)
            ws = list(si.on_wait)
            if not ws:
                continue
            kept = []
            for w in ws:
                if w.ant_name not in nonmono and \
                        str(w.wait_mode) == "sem-ge-imm" and \
                        observed.get((eng, w.ant_name), -1) >= w.wait_value:
                    continue
                kept.append(w)
            if len(kept) > 1:
                self_sem = ENG_SEM.get(eng, "zz")
                non_self = [w for w in kept
                            if not w.ant_name.startswith(self_sem)]
                if non_self:
                    kept = non_self
            if len(kept) > 1 and type(ins).__name__ == "InstMatmult":
                nonpe = [w for w in kept if not w.ant_name.startswith("PE")]
                kept = nonpe if nonpe else kept[:1]
            si.on_wait = kept
            for w in kept:
                if w.ant_name in nonmono:
                    continue
                k = (eng, w.ant_name)
                observed[k] = max(observed.get(k, -1), w.wait_value)

    for blk in nc.m.functions[0].blocks:
        for ins in blk.instructions:
            si = ins.sync_info
            if si is None or len(si.on_wait) <= 1:
                continue
            if type(ins).__name__ != "InstDrain":
                if dbg:
                    print(f"KDBG multiwait {type(ins).__name__} "
                          f"{eng_of(ins)} {ins.name}: "
                          f"{[(w.ant_name, w.wait_value) for w in si.on_wait]}")
                nonpe = [w for w in si.on_wait
                         if not w.ant_name.startswith("PE")]
                si.on_wait = nonpe if nonpe else si.on_wait[:1]

    all_dmas = [ins for blk in nc.m.functions[0].blocks
                for ins in blk.instructions
                if type(ins).__name__ == "InstDMACopy"]
    if all_dmas:
        fin = all_dmas[-1]
        if fin.sync_info and len(fin.sync_info.on_wait) > 1:
            eng = [w for w in fin.sync_info.on_wait
                   if not w.ant_name.startswith(("DMAHW", "DMASW"))]
            if eng:
                fin.sync_info.on_wait = eng

    out_dmas = all_dmas[-1:]
    keep_lanes = set()
    for ins in out_dmas:
        for u in (ins.sync_info.on_update if ins.sync_info else []):
            if u.ant_name.startswith(("DMAHW", "DMASW")):
                keep_lanes.add(u.ant_name)
    for blk in nc.m.functions[0].blocks:
        for ins in blk.instructions:
            if type(ins).__name__ == "InstDrain" and ins.sync_info and \
                    len(ins.sync_info.on_wait) > 1:
                lane_ws = [w for w in ins.sync_info.on_wait
                           if w.ant_name in keep_lanes]
                if lane_ws:
                    ins.sync_info.on_wait = lane_ws
    for blk in nc.m.functions[0].blocks:
        bad = [i for i, ins in enumerate(blk.instructions)
               if type(ins).__name__ == "InstISA"]
        if bad:
            keep = [ins for ins in blk.instructions
                    if type(ins).__name__ != "InstISA"]
            try:
                blk.instructions = keep
            except Exception:
                for i in reversed(bad):
                    del blk.instructions[i]
    _NC_CACHE["nc"] = nc
    return nc


def _combine(outs):
    """outs: list of [128, NQ*NWIN] f32 arrays (one per core). float64 combine."""
    A = np.stack([o.reshape(128, NQ, NWIN).astype(np.float64) for o in outs])
    m = np.concatenate([_band_mask9()] * SPC, axis=1)[None, :, None, :]
    sums = (A * m).sum(axis=(0, 1, 3))
    s_sp = -sums[Q_SP]  # device accumulates ln(1-pp) = -softplus(p)
    s_tp, s_ep = sums[Q_TP], sums[Q_EP]
    s_g2p, s_g2d = sums[Q_G2P], sums[Q_G2D]
    s_a, s_b = sums[Q_SA], sums[Q_SB]
    s_ab, s_curv, s_hgt = sums[Q_AB], sums[Q_CURV], sums[Q_HGT]
    mm = m[:, :, 0, :]
    min_g2p = np.where(mm > 0, A[:, :, Q_MINP, :], FBIG).min()
    max_g2p = np.where(mm > 0, A[:, :, Q_MAXP, :], -FBIG).max()
    min_g2d = np.where(mm > 0, A[:, :, Q_MIND, :], FBIG).min()
    max_g2d = np.where(mm > 0, A[:, :, Q_MAXD, :], -FBIG).max()

    bce1 = (s_sp - s_tp) / NTOT
    bce2 = (s_sp - s_ep) / NTOT

    e_a2 = s_g2p / NTOT + EPS
    e_b2 = s_g2d / NTOT + EPS
    amin, amax = np.sqrt(min_g2p + EPS), np.sqrt(max_g2p + EPS)
    bmin, bmax = np.sqrt(min_g2d + EPS), np.sqrt(max_g2d + EPS)

    def scale_off(lo, hi):
        if hi > lo:
            sc = 1.0 / (hi - lo + EPS)
            return sc, lo * sc
        return 1.0, 0.0

    sa, oa = scale_off(amin, amax)
    sb, ob = scale_off(bmin, bmax)
    cc = oa - ob
    e_a, e_b, e_ab = s_a / NTOT, s_b / NTOT, s_ab / NTOT
    grad_cons = (sa * sa * e_a2 + sb * sb * e_b2 + cc * cc
                 - 2.0 * sa * sb * e_ab - 2.0 * cc * sa * e_a
                 + 2.0 * cc * sb * e_b)

    height_cons = -s_hgt / NTOT
    curv_cons = -s_curv / NTOT
    geo = grad_cons + 0.5 * height_cons + 0.3 * curv_cons
    total = 0.8 * bce1 + 0.1 * bce2 + 0.1 * geo
    return np.float32(total)


_CONSTS = {}
_ONES128 = np.ones((128, 128), np.float32)


def kernel(pred, target, dem, _profile=False):
    from concourse.bass_utils import run_bass_kernel_spmd

    if "c" not in _CONSTS:
        _CONSTS["c"] = _build_consts()
        _CONSTS["bm"] = _band_mask9()
    cbf16 = _CONSTS["c"]
    bmask = _CONSTS["bm"]
    nc = _build_nc()

    p = np.ascontiguousarray(pred.reshape(B, H, W), dtype=np.float32)
    t = np.ascontiguousarray(target.reshape(B, H, W), dtype=np.float32)
    d = np.ascontiguousarray(dem.reshape(B, H, W), dtype=np.float32)
    in_maps = []
    for c in range(NCORES):
        sl = slice(c * SPC, (c + 1) * SPC)
        in_maps.append({
            "pred": p[sl], "target": t[sl], "dem": d[sl],
            "cbf16": cbf16, "onesf": _ONES128, "bmask": bmask,
        })
    res = run_bass_kernel_spmd(nc, in_maps, core_ids=list(range(NCORES)),
                               trace=_profile)
    outs = [m["out"] for m in res.results]
    total = _combine(outs)
    if _profile:
        return total, res
    return total
